# revision 85
# baseline (speedup 1.0000x reference)
"""Trainium2 Bass kernel for nn.DecoderBlock (pre-LN GPT block).

Shapes: B=8, T=1024, D=768, H=12, HD=64, F=3072.  Data-parallel: batch
element b runs on core b (no collectives needed).  All activations are
feature-major on chip ([D, T]: features on partitions, tokens free) so
chained matmuls need no transposes; attention scores are computed
transposed (S^T[t, q]) so softmax-weighted probabilities land directly in
the layout the P@V matmul needs.

Scheduling (the performance core of this kernel — ~1.6x over the naive
ordering of the same matmuls):
  * No GpSimd anywhere: partition broadcasts (LN apply, softmax
    denominator) are K=1 rank-1 PE matmuls into PSUM; causal masks are
    DVE multiplies on the exp output.  GpSimd semaphore+op latency
    (~1.3us per op) previously serialized the whole attention phase.
  * One ACT exp instruction per key-chunk covers BOTH heads of a
    128-partition pair via a two-bank PSUM score tile [128, 2, TQ]
    (halves ACT instruction-overhead; ACT is the attention-phase
    co-bottleneck with the PE).
  * P@V matmuls are software-pipelined two steps behind the score
    matmuls, with next-pair QK-projection matmuls split into 2-matmul
    micro-chunks pumped between steps, so the exp latency never idles
    the PE (PE idle gaps > ~3.4us re-throttle the PE clock to 1.2GHz —
    HAM — which was worth ~170us in the naive schedule).
  * V for ALL heads is computed upfront in 8 token-chunk units with the
    weight matrix as the moving operand (N=512/256 instead of N=128).
  * MLP is software-pipelined: ph(fc+1) matmuls are emitted before
    cp(fc), so each gelu runs under the next block of matmuls; final
    residual adds + output DMAs interleave into the last cp group.
  * LayerNorm: squares and PSUM evacuations on DVE; per-token stat
    chains run 128-lane in token-major layout (PE transposes); the
    apply uses rank-1 PE broadcasts.  All four ln_finish chains are
    generators whose PE ops pump between independent matmuls (stats(1)
    covers finish(0) at startup; V/QK units cover finish(1); the
    proj(1) units cover LN2's finish(0); pre-MLP fc units cover LN2's
    finish(1)), so their DVE serial chains never idle the PE.
  * LN2 runs entirely under the attention/proj tail: tci0 stats are
    last-pair filler, tci0's finish pumps into proj(1), tci1 mu AND
    sumsq stats interleave per proj(1) unit (separate PSUM banks so
    both groups stay open), and the first four fc units of the MLP run
    in the attention PSUM pools while finish(1) completes.
  * Several stat rows share one PSUM bank (matmul start=True clears
    only the bank's has_written bits, not its data), so the whole LN
    stat machinery needs just 3 banks across both layernorms.
  * Softmax normalization: the per-query denominator comes free as a
    65th ones-column in the P@V matmul; it is broadcast across
    partitions by a rank-1 PE matmul and reciprocated at full 128-lane
    DVE width after the broadcast (a [1,1024] single-lane recip is
    ~3x slower).
  * Host-side: LN affines and the 1/sqrt(HD) score scale are folded
    into the weights; weights are packed into DMA-contiguous lhsT
    tiles; matmuls run in bf16 with fp32 PSUM accumulation (fp8 was
    measured in simulation to breach the 2e-2 error budget: e4m3
    rounding alone contributes ~2.1% on the fc matmul).
  * Softmax max-subtraction is skipped: post-LN scores are O(5) so fp32
    exp cannot overflow.
  * The whole residual stream is bf16: x arrives bf16 from the host
    (kills the startup f32 DMA + on-chip cast; halves input DMA), X1 is
    bf16 (kills the LN2 scratch copies), the output leaves bf16 and is
    upcast on the host (halves output DMA, shortens the tail).
  * rstd = 1/sqrt(var+eps) runs entirely on DVE (bit-trick seed + 1
    Newton step, ~0.18% worst-case) instead of ACT Ln+Exp: each of the
    4 ln_finish calls previously triggered TWO ~1.3us ACT table loads
    on the critical path; now only the exp set (preloaded by a dummy at
    t=0) and the gelu set (preloaded right after proj(1)) are ever
    loaded, both off the PE critical path.
  * HAM warmup: ~100 junk matmuls on a memset tile run during the
    ~13us x-DMA wait so the LN1 stats chain starts at 2.4GHz instead
    of the cold 1.2GHz clock.
  * Softmax normalization of each (pair, qc) is deferred until the
    next score batch is in the PE queue, so its DVE work (den2 evac,
    recip, YT scale) overlaps scores instead of gating them.
  * Final residual adds are folded into the cp PSUM accumulation as
    identity matmuls; the output evacuations alternate DVE/ACT and the
    bf16 output DMA is half the size, shortening the post-last-matmul
    tail to ~6us.
"""

import numpy as np
import ml_dtypes

import concourse.bass as bass
import concourse.mybir as mybir
import concourse.tile as tile
from concourse import bacc

BF16 = mybir.dt.bfloat16
F32 = mybir.dt.float32
AF = mybir.ActivationFunctionType
OP = mybir.AluOpType

B, T, D, H = 8, 1024, 768, 12
HD = D // H
F = 4 * D
EPS = 1e-5
N_CORES = 8

KC = D // 128   # 6  contraction chunks over D
MC = D // 128   # 6  output-feature chunks over D
FC = F // 128   # 24 chunks over MLP hidden
NT = T // 128   # 8  key chunks
TS = 512        # token chunk (LN/proj/MLP)
TQ = 512        # query chunk
NQ = T // TQ    # 2
NJ = TS // 128  # 4
HPC = 2         # heads per 128-partition group
NPAIR = H // HPC  # 6
VS = HD + 1     # V columns per head incl ones-column (denominator row)


def build_decoder_nc(with_bias=False, eps=EPS, gelu_func=AF.Gelu_apprx_tanh):
    scale = 1.0  # 1/sqrt(HD) folded into wq host-side

    nc = bacc.Bacc()

    xT = nc.dram_tensor("xT", [D, T], BF16, kind="ExternalInput")
    wq_d = nc.dram_tensor("wq", [MC, 128, KC, 128], BF16, kind="ExternalInput")
    wk_d = nc.dram_tensor("wk", [MC, 128, KC, 128], BF16, kind="ExternalInput")
    wv_d = nc.dram_tensor("wv", [128, KC, D], BF16, kind="ExternalInput")
    wp_d = nc.dram_tensor("wp", [MC, 128, KC, 128], BF16, kind="ExternalInput")
    wf_d = nc.dram_tensor("wf", [FC, 128, KC, 128], BF16, kind="ExternalInput")
    wc_d = nc.dram_tensor("wc", [FC, 128, MC, 128], BF16, kind="ExternalInput")
    bias_d = {}
    if with_bias:
        for nm, width in (("bq", D), ("bk", D), ("bv", D), ("bp", D),
                          ("bf", F), ("bc", D)):
            bias_d[nm] = nc.dram_tensor(nm, [1, width], BF16,
                                        kind="ExternalInput")
    outT = nc.dram_tensor("outT", [D, T], BF16, kind="ExternalOutput")
    outT_t = outT[:].rearrange("(o p) t -> p o t", p=128)

    # ---- constants ----
    ones_bf = nc.inline_tensor(np.ones((1, T), ml_dtypes.bfloat16), "ones_bf")
    onescol_f = nc.inline_tensor(np.ones((128, 1), np.float32), "onescol_f")
    onescol_b = nc.inline_tensor(np.ones((128, 1), ml_dtypes.bfloat16),
                                 "onescol_b")
    onesrow_f = nc.inline_tensor(np.ones((1, 128), np.float32), "onesrow_f")
    onesrow_b = nc.inline_tensor(np.ones((1, 128), ml_dtypes.bfloat16),
                                 "onesrow_b")
    ident_b = nc.inline_tensor(np.eye(128, dtype=ml_dtypes.bfloat16),
                               "ident_b")
    # sel2[h, m] = 1 if m // 64 == h  (half-select broadcast)
    sel2_np = (np.arange(128)[None, :] // 64 ==
               np.arange(2)[:, None]).astype(np.float32)
    sel2_c = nc.inline_tensor(sel2_np, "sel2_c")
    # sel4[k, m] = 1 if m // 128 == k  (ln-stat row-select broadcast)
    sel4_np = (np.arange(512)[None, :] // 128 ==
               np.arange(4)[:, None]).astype(ml_dtypes.bfloat16)
    sel4_c = nc.inline_tensor(sel4_np, "sel4_c")
    ident_c = nc.inline_tensor(np.eye(128, dtype=np.float32), "ident_c")
    # multiplicative causal mask for transposed diagonal blocks: 1 if t <= q
    m_np = (np.arange(128)[:, None] <= np.arange(128)[None, :]).astype(
        ml_dtypes.bfloat16)
    masks_d = nc.inline_tensor(m_np, "masks")

    with tile.TileContext(nc) as tc:
        with (
            tc.tile_pool(name="persist", bufs=1) as pp,
            tc.tile_pool(name="wts", bufs=4) as wpool,
            tc.tile_pool(name="work", bufs=4) as wkp,
            tc.tile_pool(name="small", bufs=1) as sp,
        ):
            # ---- persistent SBUF ----
            ALN = pp.tile([128, KC, T], BF16, tag="ALN", name="ALN")
            QT = pp.tile([128, MC, T], BF16, tag="QT", name="QT")
            KT = pp.tile([128, MC, T], BF16, tag="KT", name="KT")
            Vt = pp.tile([128, NT, H * VS], BF16, tag="Vt", name="Vt")
            YT = pp.tile([128, KC, T], BF16, tag="YT", name="YT")
            X1 = pp.tile([128, KC, T], BF16, tag="X1", name="X1")
            A2 = pp.tile([128, KC, T], BF16, tag="A2", name="A2")
            wv_s = pp.tile([128, KC, D], BF16, tag="wv", name="wv_s")

            onescol_fs = pp.tile([128, 1], F32, tag="oc_f", name="onescol_fs")
            onescol_bs = pp.tile([128, 1], BF16, tag="oc_b", name="onescol_bs")
            onesrow_fs = pp.tile([1, 128], F32, tag="or_f", name="onesrow_fs")
            onesrow_bs = pp.tile([1, 128], BF16, tag="or_b", name="onesrow_bs")
            ident_bs = pp.tile([128, 128], BF16, tag="id_b", name="ident_bs")
            sel2_s = pp.tile([2, 128], F32, tag="sel2", name="sel2_s")
            sel4_s = pp.tile([4, 512], BF16, tag="sel4", name="sel4_s")
            ident_s = pp.tile([128, 128], F32, tag="ident", name="ident_s")
            masks_s = pp.tile([128, 128], BF16, tag="masks", name="masks_s")
            nc.sync.dma_start(out=onescol_fs, in_=onescol_f[:])
            nc.sync.dma_start(out=onescol_bs, in_=onescol_b[:])
            nc.sync.dma_start(out=onesrow_fs, in_=onesrow_f[:])
            nc.sync.dma_start(out=onesrow_bs, in_=onesrow_b[:])
            nc.sync.dma_start(out=ident_bs, in_=ident_b[:])
            nc.sync.dma_start(out=sel2_s, in_=sel2_c[:])
            nc.sync.dma_start(out=sel4_s, in_=sel4_c[:])
            nc.sync.dma_start(out=ident_s, in_=ident_c[:])
            nc.sync.dma_start(out=masks_s, in_=masks_d[:])
            onesb_s = None
            biases = {}
            if with_bias:
                onesb_s = pp.tile([1, T], BF16, tag="onesb", name="onesb_s")
                nc.sync.dma_start(out=onesb_s, in_=ones_bf[:])
                for nm, dten in bias_d.items():
                    bt = pp.tile(list(dten.shape), BF16, tag=nm,
                                 name=f"{nm}_s")
                    nc.sync.dma_start(out=bt, in_=dten[:])
                    biases[nm] = bt

            def bias_mm(psum, bias_t, msl, tsl):
                if bias_t is None:
                    return True
                nc.tensor.matmul(psum, bias_t[0:1, msl], onesb_s[0:1, tsl],
                                 start=True, stop=False)
                return False

            # ================= attention-phase PSUM pools =================
            with (
                tc.tile_pool(name="psA", bufs=2, space="PSUM") as psA,  # 4 banks
                tc.tile_pool(name="psB", bufs=1, space="PSUM") as psB,  # 2 banks
                tc.tile_pool(name="psC", bufs=2, space="PSUM") as psC,  # 2 banks
            ):
                # ---- load x^T (bf16 from host) BEFORE the V weights: the
                # LN1 stats chain gates everything, and per-kc full-T chunks
                # keep 2KB DMA lines (a tci-split halves line size and DMA
                # efficiency) ----
                xT_t = xT[:].rearrange("(o p) t -> p o t", p=128)
                for kc in range(KC):
                    nc.sync.dma_start(out=A2[:, kc, :], in_=xT_t[:, kc, :])
                nc.sync.dma_start(out=wv_s, in_=wv_d[:])

                # ---- LayerNorm (stats + apply); bf16 src, bf16 dst ----
                # row_mu / row_sq are [1, TS] PSUM APs.  A matmul start=True
                # zeroes the has_written bits of the whole 2KB bank, but not
                # the data, so many stat rows can share a bank as long as
                # their accumulation groups are strictly sequential.
                def ln_stats(sb, tci, row_mu, row_sq, sq, sqsl=None):
                    tsl = slice(tci * TS, (tci + 1) * TS)
                    for kc in range(KC):
                        nc.tensor.matmul(row_mu, onescol_bs[:],
                                         sb[:, kc, tsl],
                                         start=(kc == 0), stop=(kc == KC - 1))
                    for kc in range(KC):
                        nc.tensor.matmul(row_sq, onescol_bs[:],
                                         sq[:, kc, sqsl or slice(None)],
                                         start=(kc == 0), stop=(kc == KC - 1))

                def ln_finish_gen(src, dst, tci, row_mu, row_sq,
                                  act_evac=True, prep_pool=None,
                                  coarse_apply=False):
                    tsl = slice(tci * TS, (tci + 1) * TS)
                    srow_mu = sp.tile([1, TS], F32, tag="srow_mu",
                                      name="srow_mu")
                    srow_sq = sp.tile([1, TS], F32, tag="srow_sq",
                                      name="srow_sq")
                    # evacuate the two stat rows on different engines so they
                    # run concurrently (GpSimd cannot read PSUM); near the
                    # attention->MLP seam ACT is backlogged with exp/gelu, so
                    # those call sites use DVE for both
                    if act_evac:
                        nc.scalar.copy(out=srow_mu, in_=row_mu)
                    else:
                        nc.vector.tensor_copy(out=srow_mu, in_=row_mu)
                    nc.vector.tensor_copy(out=srow_sq, in_=row_sq)
                    yield
                    # token-major [128, NJ, 2] via PE transposes
                    ptk = psC.tile([128, NJ, 2], F32, tag="psC", name="ptk")
                    for jj in range(NJ):
                        jsl = slice(jj * 128, (jj + 1) * 128)
                        nc.tensor.transpose(
                            ptk[:, jj, 0:1], srow_mu[:, jsl],
                            ident_s[0:1, 0:1])
                        nc.tensor.transpose(
                            ptk[:, jj, 1:2], srow_sq[:, jsl],
                            ident_s[0:1, 0:1])
                        yield
                    stok = sp.tile([128, NJ, 2], F32, tag="stok", name="stok")
                    nc.vector.tensor_copy(out=stok, in_=ptk)
                    nc.vector.tensor_scalar_mul(stok, stok, 1.0 / D)
                    mu = stok[:, :, 0]
                    m2 = stok[:, :, 1]
                    var_t = sp.tile([128, NJ], F32, tag="var_t", name="var_t")
                    nc.vector.tensor_tensor(var_t, mu, mu, OP.mult)
                    nc.vector.tensor_tensor(var_t, m2, var_t, OP.subtract)
                    nc.vector.tensor_scalar_add(var_t, var_t, eps)
                    # st2[:,0,:]=rstd  st2[:,1,:]=-mu*rstd
                    # rstd = 1/sqrt(var+eps) entirely on DVE: bit-trick seed
                    # + 2 Newton steps.  No ACT Ln/Exp -> no ~1.3us ACT table
                    # loads on the LN critical path (2 per finish before).
                    st2 = sp.tile([128, 2, NJ], F32, tag="st2", name="st2")
                    y = st2[:, 0, :]
                    nr = sp.tile([128, NJ], F32, tag="nr_tmp", name="nr_tmp")
                    nc.vector.tensor_scalar(
                        y.bitcast(mybir.dt.int32),
                        var_t[:, :].bitcast(mybir.dt.int32),
                        1, None, OP.logical_shift_right)
                    # magic - t  ==  ~t + (magic + 1)  (two's complement);
                    # bitwise and arith ops can't share one instruction
                    nc.vector.tensor_scalar(
                        y.bitcast(mybir.dt.int32), y.bitcast(mybir.dt.int32),
                        0, None, OP.bitwise_not)
                    nc.vector.tensor_scalar(
                        y.bitcast(mybir.dt.int32), y.bitcast(mybir.dt.int32),
                        0x5f375a86 + 1, None, OP.add)
                    # one Newton step: <=0.18% rstd error, invisible next to
                    # the bf16 noise floor, and ~0.7us less serial DVE per
                    # ln_finish chain
                    for _ in range(1):
                        nc.vector.tensor_tensor(nr, y, y, OP.mult)
                        nc.vector.tensor_tensor(nr, nr, var_t, OP.mult)
                        nc.vector.tensor_scalar(nr, nr, -0.5, 1.5,
                                                OP.mult, OP.add)
                        nc.vector.tensor_tensor(y, y, nr, OP.mult)
                    nc.vector.tensor_tensor(st2[:, 1, :], mu, y, OP.mult)
                    nc.vector.tensor_scalar_mul(st2[:, 1, :], st2[:, 1, :],
                                                -1.0)
                    yield
                    # back to row layout per stat (all base-partition-0 APs)
                    prow = psC.tile([NJ, 2, 128], F32, tag="psC", name="prow")
                    nc.tensor.transpose(prow[:, 0, :], st2[:, 0, :],
                                        ident_s[:])
                    nc.tensor.transpose(prow[:, 1, :], st2[:, 1, :],
                                        ident_s[:])
                    yield
                    rows_sb = sp.tile([NJ, 2, 128], BF16, tag="rows_sb",
                                      name="rows_sb")
                    nc.vector.tensor_copy(out=rows_sb, in_=prow)
                    yield
                    # broadcast each 128-token stat row across partitions via
                    # a K=4 row-select matmul -- replaces a ~1.7us SBUF
                    # gather DMA that used to sit on this chain
                    pool = prep_pool if prep_pool is not None else psA
                    prep = pool.tile([128, 2, TS], F32, tag=pool.name,
                                     name="prep")
                    for jj in range(NJ):
                        jsl = slice(jj * 128, (jj + 1) * 128)
                        for sti in range(2):
                            nc.tensor.matmul(
                                prep[:, sti, jsl], sel4_s[:, jsl],
                                rows_sb[:, sti, :],
                                start=True, stop=True)
                    yield
                    tmp = wkp.tile([128, KC, TS], BF16, tag="lntmp", bufs=1,
                                   name="lntmp")
                    if coarse_apply:
                        # one whole-tci mult + add: consumers need the full
                        # half anyway, and 2 big DVE ops retire ~2x sooner
                        # than 8 per-jj ones (less per-op overhead + queue)
                        nc.vector.tensor_tensor(
                            tmp, src[:, :, tsl],
                            prep[:, 0:1, :].to_broadcast((128, KC, TS)),
                            OP.mult)
                        nc.vector.tensor_tensor(
                            dst[:, :, tsl], tmp,
                            prep[:, 1:2, :].to_broadcast((128, KC, TS)),
                            OP.add)
                        return
                    for jj in range(NJ):
                        jsl = slice(jj * 128, (jj + 1) * 128)
                        jtl = slice(tci * TS + jj * 128,
                                    tci * TS + (jj + 1) * 128)
                        # jj0 in two kc-halves: the first V/QK consumer can
                        # start on kc 0-2 ~1us before the full chunk lands
                        khs = ([slice(0, KC // 2), slice(KC // 2, KC)]
                               if jj == 0 else [slice(0, KC)])
                        for kh in khs:
                            nkc = kh.stop - kh.start
                            nc.vector.tensor_tensor(
                                tmp[:, kh, jsl], src[:, kh, jtl],
                                prep[:, 0:1, jsl].to_broadcast(
                                    (128, nkc, 128)), OP.mult)
                            nc.vector.tensor_tensor(
                                dst[:, kh, jtl], tmp[:, kh, jsl],
                                prep[:, 1:2, jsl].to_broadcast(
                                    (128, nkc, 128)), OP.add)

                def ln_finish(src, dst, tci, row_mu, row_sq, act_evac=True):
                    for _ in ln_finish_gen(src, dst, tci, row_mu, row_sq,
                                           act_evac):
                        pass

                # ---- V units (one per 128-token chunk, all heads) ----
                Vt4 = Vt.rearrange("p t (h c) -> p t h c", c=VS)
                nc.vector.memset(Vt4[:, :, :, HD:HD + 1], 1.0)

                def v_unit(tch):
                    t128 = slice(tch * 128, (tch + 1) * 128)
                    pvv = psA.tile([128, 2, TS], F32, tag="psA", name="pvv")
                    st0 = True
                    st1 = True
                    if with_bias:
                        nc.tensor.matmul(pvv[:, 0, :], onesb_s[0:1, 0:128],
                                         biases["bv"][0:1, 0:512],
                                         start=True, stop=False)
                        nc.tensor.matmul(pvv[:, 1, 0:256],
                                         onesb_s[0:1, 0:128],
                                         biases["bv"][0:1, 512:768],
                                         start=True, stop=False)
                        st0 = st1 = False
                    for kc in range(KC):
                        nc.tensor.matmul(
                            pvv[:, 0, :], ALN[:, kc, t128],
                            wv_s[:, kc, 0:512],
                            start=st0 and (kc == 0), stop=(kc == KC - 1))
                        nc.tensor.matmul(
                            pvv[:, 1, 0:256], ALN[:, kc, t128],
                            wv_s[:, kc, 512:768],
                            start=st1 and (kc == 0), stop=(kc == KC - 1))
                    # split the evacuation across ACT and DVE
                    nc.scalar.copy(
                        out=Vt4[:, tch, 0:8, 0:HD],
                        in_=pvv[:, 0, :].rearrange("p (h c) -> p h c", c=HD))
                    nc.vector.tensor_copy(
                        out=Vt4[:, tch, 8:12, 0:HD],
                        in_=pvv[:, 1, 0:256].rearrange("p (h c) -> p h c",
                                                       c=HD))

                # ---- QK unit generators (pair mc); yield per 2 matmuls ----
                def make_qk_gens(mc, tcis=(0, 1), on_act=False):
                    msl = slice(mc * 128, (mc + 1) * 128)
                    gens = []
                    wts = []
                    for nm, wten, dstT in (("bq", wq_d, QT), ("bk", wk_d, KT)):
                        wt = wpool.tile([128, KC, 128], BF16, tag="w_qk",
                                        bufs=4, name="wt_qk")
                        nc.sync.dma_start(out=wt, in_=wten[mc])
                        wts.append((nm, wt, dstT))
                    for tci in tcis:
                        for nm, wt, dstT in wts:
                            def qk_gen(nm=nm, wt=wt, dstT=dstT, tci=tci,
                                       on_act=on_act):
                                tsl = slice(tci * TS, (tci + 1) * TS)
                                pq = psC.tile([128, TS], F32, tag="psC",
                                              name="pq")
                                st = bias_mm(pq, biases.get(nm), msl, tsl)
                                for kc in range(KC):
                                    nc.tensor.matmul(
                                        pq, wt[:, kc, :], ALN[:, kc, tsl],
                                        start=st and (kc == 0),
                                        stop=(kc == KC - 1))
                                    if kc % 2 == 1 and kc < KC - 1:
                                        yield
                                if on_act:
                                    nc.scalar.copy(out=dstT[:, mc, tsl],
                                                   in_=pq)
                                else:
                                    nc.vector.tensor_copy(
                                        out=dstT[:, mc, tsl], in_=pq)
                            gens.append(qk_gen())
                    return gens

                # ---- proj unit generators (attn out-proj + residual) ----
                def make_proj_gens(tci):
                    tsl = slice(tci * TS, (tci + 1) * TS)
                    gens = []
                    for mc in range(MC):
                        wt = wpool.tile([128, KC, 128], BF16, tag="w_p",
                                        bufs=4, name="wt_p")
                        nc.sync.dma_start(out=wt, in_=wp_d[mc])

                        def proj_gen(mc=mc, wt=wt):
                            msl = slice(mc * 128, (mc + 1) * 128)
                            po = psC.tile([128, TS], F32, tag="psC", name="po")
                            st = bias_mm(po, biases.get("bp"), msl, tsl)
                            for kc in range(KC):
                                nc.tensor.matmul(
                                    po, wt[:, kc, :], YT[:, kc, tsl],
                                    start=st and (kc == 0), stop=False)
                                if kc % 2 == 1 and kc < KC - 1:
                                    yield
                            # residual folded into the PE accumulation
                            # (identity @ X); the X1 evacuation then runs on
                            # the idle ACT instead of the congested DVE
                            nc.tensor.matmul(
                                po, ident_bs[:], A2[:, mc, tsl],
                                start=False, stop=True)
                            nc.scalar.copy(out=X1[:, mc, tsl], in_=po)
                        gens.append(proj_gen(mc, wt))
                    return gens

                def pump(gens, n):
                    """Advance the generator queue by n yield-chunks."""
                    while n > 0 and gens:
                        try:
                            next(gens[0])
                        except StopIteration:
                            gens.pop(0)
                            continue
                        n -= 1

                def drain(gens):
                    for g in gens:
                        for _ in g:
                            pass
                    gens.clear()

                # ---- startup: LN1 overlapped with V and pair-0 QK ----
                # HAM warmup: the PE clock starts throttled at 1.2GHz and
                # only unthrottles after ~3.4us of sustained activity.  The
                # x DMA takes ~13us to land, so burn that wait on junk
                # matmuls (memset source: no DMA dependency) to both warm
                # the clock and keep it warm until the stats arrive.
                wu_src = sp.tile([128, 128], BF16, tag="wu_src", name="wu_src")
                nc.vector.memset(wu_src, 1.0)
                # trigger the exp table-set load NOW: it covers Copy too, so
                # ACT never loads a table again until the MLP gelu
                wu_act = sp.tile([1, 128], F32, tag="wu_act", name="wu_act")
                nc.scalar.activation(out=wu_act, in_=wu_src[0:1, :],
                                     func=AF.Exp)
                junk = psA.tile([128, 2, TQ], F32, tag="psA", name="junk")
                for _ in range(100):
                    nc.tensor.matmul(junk[:, 0, 0:128], wu_src[:],
                                     wu_src[:], start=True, stop=True)
                # x squares for BOTH token halves upfront: DVE computes them
                # chunk-by-chunk as the x DMA lands, while the PE is still on
                # warmup junk -- so the DVE is free later when the ln_finish
                # chains need it
                sq_all = wkp.tile([128, KC, T], BF16, tag="sq_all", bufs=1,
                                  name="sq_all")
                for kc in range(KC):
                    nc.vector.tensor_tensor(sq_all[:, kc, :], A2[:, kc, :],
                                            A2[:, kc, :], OP.mult)
                pstat1 = psB.tile([128, 2, TS], F32, tag="psB", name="pstat1")
                ln_stats(A2, 0, pstat1[0:1, 0, :], pstat1[0:1, 1, :],
                         sq_all, slice(0, TS))
                # finish(0) must fully drain before any ALN(tci0) consumer
                # matmul is emitted (the in-order PE queue would deadlock);
                # the stats(1) matmuls are pumped in as its latency cover.
                fin0 = [ln_finish_gen(A2, ALN, 0, pstat1[0:1, 0, :],
                                      pstat1[0:1, 1, :])]
                pump(fin0, 1)  # stat-row evacuations start now
                ln_stats(A2, 1, pstat1[64:65, 0, :], pstat1[64:65, 1, :],
                         sq_all, slice(TS, T))
                drain(fin0)
                # fin1's prep goes in psB (pstat1's slot, dead after its own
                # evacuations): in psA it would join the pvv rotation and
                # stall v_unit(4) on the whole LN1 apply(1)
                qk0_a = make_qk_gens(0, tcis=(0,), on_act=True)
                qk0_a.append(ln_finish_gen(A2, ALN, 1, pstat1[64:65, 0, :],
                                           pstat1[64:65, 1, :],
                                           prep_pool=psB))
                for tch in range(4):
                    v_unit(tch)
                    pump(qk0_a, 6)
                drain(qk0_a)
                qk0_b = make_qk_gens(0, tcis=(1,), on_act=True)
                for tch in range(4, NT):
                    v_unit(tch)
                    pump(qk0_b, 3)
                drain(qk0_b)
                # LN2(tci0) stats + finish as last-pair filler generators:
                # pumped after proj(0) drains, so apply(0) runs on DVE while
                # attention finishes and MLP can start right after proj(1).
                # pstat0 is allocated lazily (at first pump) so the psC
                # rotation during earlier pairs cannot clobber it.
                pstat0_box = []

                def stats0_gen():
                    pstat0_box.append(
                        psC.tile([128, TS], F32, tag="psC", name="pstat0"))
                    pstat0 = pstat0_box[0]
                    tsl0 = slice(0, TS)
                    sq = wkp.tile([128, KC, TS], BF16, tag="sq", bufs=1,
                                  name="sq")
                    for kc in range(KC):
                        nc.vector.tensor_tensor(
                            sq[:, kc, :], X1[:, kc, tsl0], X1[:, kc, tsl0],
                            OP.mult)
                    for kc in range(KC):
                        nc.tensor.matmul(pstat0[0:1, :], onescol_bs[:],
                                         X1[:, kc, tsl0],
                                         start=(kc == 0), stop=(kc == KC - 1))
                        if kc % 2 == 1 and kc < KC - 1:
                            yield
                    for kc in range(KC):
                        nc.tensor.matmul(pstat0[32:33, :], onescol_bs[:],
                                         sq[:, kc, :],
                                         start=(kc == 0), stop=(kc == KC - 1))
                        if kc % 2 == 1:
                            yield

                # finish(0)-of-LN2 starts inside the last-pair filler: its
                # stat-row evacuations then sit in the DVE queue AHEAD of the
                # last normalize/square batch instead of behind it
                fin0_box = []

                def fin0_start():
                    p0 = pstat0_box[0]
                    g = ln_finish_gen(X1, A2, 0, p0[0:1, :], p0[32:33, :],
                                      act_evac=False, coarse_apply=True)
                    fin0_box.append(g)
                    next(g)
                    yield

                norm_pend = []
                for mc in range(NPAIR):
                    last = mc + 1 >= NPAIR
                    filler = (make_proj_gens(0) if last
                              else make_qk_gens(mc + 1))
                    if last:
                        filler.append(stats0_gen())
                        filler.append(fin0_start())
                    for qc in range(NQ):
                        qsl = slice(qc * TQ, (qc + 1) * TQ)
                        ntch = (qc + 1) * (TQ // 128)
                        py = psB.tile([128, 2, TQ], F32, tag="psB", name="py")
                        pv_pend = []

                        def emit_pv(tch, pexp, rq, ntch=ntch, py=py, mc=mc):
                            for half in range(HPC):
                                h = mc * HPC + half
                                nc.tensor.matmul(
                                    py[0:VS, half, rq],
                                    Vt[:, tch, h * VS:(h + 1) * VS],
                                    pexp[:, half, rq],
                                    start=(tch == 0), stop=(tch == ntch - 1))

                        for tch in range(ntch):
                            t128 = slice(tch * 128, (tch + 1) * 128)
                            diag0 = qc * (TQ // 128)
                            dq = max(0, tch - diag0) * 128
                            rq = slice(dq, TQ)
                            qslr = slice(qc * TQ + dq, (qc + 1) * TQ)
                            psc = psA.tile([128, 2, TQ], F32, tag="psA",
                                           name="psc")
                            for half in range(HPC):
                                hsl = slice(half * HD, (half + 1) * HD)
                                nc.tensor.matmul(
                                    psc[:, half, rq], KT[hsl, mc, t128],
                                    QT[hsl, mc, qslr], start=True, stop=True)
                            # previous (pair, qc)'s softmax-normalize runs
                            # here, after this qc's first scores are already
                            # in the PE queue: its DVE work overlaps the PE
                            # instead of gating it at the qc boundary
                            if tch == 1 and norm_pend:
                                norm_pend.pop(0)()
                            pexp = wkp.tile([128, 2, TQ], BF16, tag="pexp",
                                            bufs=5, name="pexp")
                            nc.scalar.activation(out=pexp[:, :, rq],
                                                 in_=psc[:, :, rq],
                                                 func=AF.Exp)
                            if tch >= diag0:
                                nc.vector.tensor_tensor(
                                    pexp[:, :, dq:dq + 128],
                                    pexp[:, :, dq:dq + 128],
                                    masks_s[:, None, :].to_broadcast(
                                        (128, 2, 128)), OP.mult)
                            pv_pend.append((tch, pexp, rq))
                            # filler micro-chunks BEFORE the PV: the PE queue
                            # is in-order, so a PV stalled on its exp must
                            # not trap independent filler matmuls behind it
                            if qc == NQ - 1 or not last:
                                pump(filler, 4 if last else 1)
                            # software pipeline: PV three steps behind
                            # scores so the exp latency never stalls the PE
                            if len(pv_pend) > 3:
                                emit_pv(*pv_pend.pop(0))
                        if qc == NQ - 1 or not last:
                            pump(filler, 2)
                        while pv_pend:
                            emit_pv(*pv_pend.pop(0))

                        # softmax normalization (no GpSimd): both halves'
                        # denominators side by side in one base-0 row
                        def normalize(py=py, qsl=qsl, mc=mc):
                            den2 = sp.tile([1, 2 * TQ], BF16, tag="den2",
                                           name="den2")
                            for half in range(HPC):
                                nc.vector.tensor_copy(
                                    out=den2[:, half * TQ:(half + 1) * TQ],
                                    in_=py[HD:HD + 1, half, :])
                            prep1 = psC.tile([128, TS], F32, tag="psC",
                                             name="prep1")
                            for half in range(HPC):
                                nc.tensor.matmul(
                                    prep1[half * HD:(half + 1) * HD, :],
                                    onesrow_bs[0:1, 0:HD],
                                    den2[:, half * TQ:(half + 1) * TQ],
                                    start=True, stop=True)
                            # reciprocal AFTER the broadcast: 128-lane DVE op
                            # instead of a slow single-partition recip
                            prep_sb = wkp.tile([128, TS], F32, tag="prep_sb",
                                               bufs=2, name="prep_sb")
                            nc.vector.reciprocal_approx_fast(out=prep_sb,
                                                             in_=prep1)
                            for half in range(HPC):
                                hsl = slice(half * HD, (half + 1) * HD)
                                nc.vector.tensor_tensor(
                                    YT[hsl, mc, qsl], py[0:HD, half, :],
                                    prep_sb[hsl, :], OP.mult)

                        if last and qc == 0:
                            # proj(0) filler reads YT(tci0) this pair's qc=1:
                            # cannot defer
                            normalize()
                        else:
                            norm_pend.append(normalize)
                    drain(filler)
                while norm_pend:
                    norm_pend.pop(0)()

                # Prefetch the first MLP weight tiles now so their DMAs don't
                # queue behind the LN2 rows DMAs right when the MLP starts.
                pre_wts = []
                for fc in range(4):
                    wt = wpool.tile([128, KC, 128], BF16, tag="w_f",
                                    bufs=4, name="wt_f")
                    nc.sync.dma_start(out=wt, in_=wf_d[fc])
                    wtc = wpool.tile([128, MC, 128], BF16, tag="w_c",
                                     bufs=4, name="wt_c")
                    nc.sync.dma_start(out=wtc, in_=wc_d[fc])
                    pre_wts.append((wt, wtc))
                # LN2(tci0) stats ran as last-pair filler; finish(0) here so
                # apply(0) runs on DVE underneath proj(1).  LN2(tci1) mu AND
                # sumsq both interleave with the proj(1) units -- they
                # accumulate in different PSUM banks so both groups can stay
                # open across the loop.
                # emit finish(0)'s transposes + DVE-chain now, but leave its
                # later PE ops (prow/preps/apply) to pump between the proj(1)
                # units: the DVE chain then runs UNDER the proj matmuls
                # instead of idling the PE
                pump(fin0_box, 5)
                pstL2 = psB.tile([128, 2, TS], F32, tag="psB", name="pstL2")
                proj1 = make_proj_gens(1)
                tsl1 = slice(TS, 2 * TS)
                sq1 = wkp.tile([128, KC, TS], BF16, tag="sq1", bufs=1,
                               name="sq1")
                for mc, g in enumerate(proj1):
                    for _ in g:
                        pass
                    nc.vector.tensor_tensor(sq1[:, mc, :], X1[:, mc, tsl1],
                                            X1[:, mc, tsl1], OP.mult)
                    nc.tensor.matmul(pstL2[0:1, 0, :], onescol_bs[:],
                                     X1[:, mc, tsl1],
                                     start=(mc == 0), stop=(mc == KC - 1))
                    nc.tensor.matmul(pstL2[0:1, 1, :], onescol_bs[:],
                                     sq1[:, mc, :],
                                     start=(mc == 0), stop=(mc == KC - 1))
                    if mc >= 2:
                        pump(fin0_box, 2)
                drain(fin0_box)
                # trigger the gelu table-set load now: the proj X1
                # evacuations are done with ACT and the first real gelu
                # (pre-MLP units below) then hits a warm table.  Any earlier
                # and the 1.3us load delays a proj evacuation -> PE stall.
                nc.scalar.activation(out=wu_act, in_=wu_src[0:1, :],
                                     func=gelu_func)
                # Pre-MLP: the first two fc(qc=0) units run here in the
                # attention PSUM pools (they only need apply(0), done under
                # proj(1)), with finish(1) pumped between them, so the PE
                # never drains while LN2(tci1) finishes on DVE.
                fin1 = [ln_finish_gen(X1, A2, 1,
                                      pstL2[0:1, 0, :], pstL2[0:1, 1, :],
                                      act_evac=False, coarse_apply=True)]
                pump(fin1, 1)  # stat-row evacuations start now
                pre_hgel = []
                tsl0 = slice(0, TS)
                for fc in range(4):
                    wt, _wtc = pre_wts[fc]
                    ph = psA.tile([128, 2, TQ], F32, tag="psA",
                                  name="ph_pre")
                    st = bias_mm(ph[:, 0, 0:TS], biases.get("bf"),
                                 slice(fc * 128, (fc + 1) * 128), tsl0)
                    for kc in range(KC):
                        nc.tensor.matmul(
                            ph[:, 0, 0:TS], wt[:, kc, :], A2[:, kc, tsl0],
                            start=st and (kc == 0), stop=(kc == KC - 1))
                    hgel = wkp.tile([128, TS], BF16, tag="hgel", bufs=6,
                                    name="hgel")
                    nc.scalar.activation(out=hgel, in_=ph[:, 0, 0:TS],
                                         func=gelu_func)
                    pre_hgel.append(hgel)
                # both pre-MLP units are emitted BEFORE the rest of
                # finish(1): its PE ops wait on the DVE chain, and the fc
                # matmuls are the cover
                drain(fin1)

            # ================= MLP phase (new PSUM pools) =================
            with (
                tc.tile_pool(name="psPC", bufs=6, space="PSUM") as psPC,
                tc.tile_pool(name="psPH", bufs=2, space="PSUM") as psPH,
            ):
                for qc in range(T // TS):
                    tsl = slice(qc * TS, (qc + 1) * TS)
                    pcs = []
                    for mc in range(MC):
                        pc = psPC.tile([128, TS], F32, tag="psPC",
                                       name=f"pc{mc}")
                        st = bias_mm(pc, biases.get("bc"),
                                     slice(mc * 128, (mc + 1) * 128), tsl)
                        pcs.append((pc, st))
                    cp_pend = []
                    if qc == 0:
                        for fc in range(4):
                            cp_pend.append((fc, pre_hgel[fc], pre_wts[fc][1]))

                    def emit_cp(fc, hgel, wtc, pcs=pcs):
                        for mc in range(MC):
                            pc, st = pcs[mc]
                            nc.tensor.matmul(
                                pc, wtc[:, mc, :], hgel,
                                start=st and (fc == 0), stop=(fc == FC - 1))

                    for fc in range(len(cp_pend), FC):
                        fsl = slice(fc * 128, (fc + 1) * 128)
                        wt = wpool.tile([128, KC, 128], BF16, tag="w_f",
                                        bufs=4, name="wt_f")
                        nc.sync.dma_start(out=wt, in_=wf_d[fc])
                        wtc = wpool.tile([128, MC, 128], BF16, tag="w_c",
                                         bufs=4, name="wt_c")
                        nc.sync.dma_start(out=wtc, in_=wc_d[fc])
                        ph = psPH.tile([128, TS], F32, tag="psPH", name="ph")
                        st = bias_mm(ph, biases.get("bf"), fsl, tsl)
                        for kc in range(KC):
                            nc.tensor.matmul(
                                ph, wt[:, kc, :], A2[:, kc, tsl],
                                start=st and (kc == 0), stop=(kc == KC - 1))
                        hgel = wkp.tile([128, TS], BF16, tag="hgel", bufs=6,
                                        name="hgel")
                        nc.scalar.activation(out=hgel, in_=ph, func=gelu_func)
                        cp_pend.append((fc, hgel, wtc))
                        # deep pipeline: the first cp write of each qc lands
                        # on PSUM banks whose previous reader (LN2 apply /
                        # the other qc's residual adds, both on DVE) may
                        # still be draining
                        if len(cp_pend) > 4:
                            emit_cp(*cp_pend.pop(0))
                    while cp_pend:
                        fc_l, hgel_l, wtc_l = cp_pend.pop(0)
                        for mc in range(MC):
                            pc, st = pcs[mc]
                            nc.tensor.matmul(
                                pc, wtc_l[:, mc, :], hgel_l,
                                start=st and (fc_l == 0), stop=False)
                            if fc_l == FC - 1:
                                # residual add folded into the PE
                                # accumulation; the evacuation is then a
                                # plain copy that alternates ACT/DVE so the
                                # output tail drains on two engines
                                nc.tensor.matmul(
                                    pc, ident_bs[:], X1[:, mc, tsl],
                                    start=False, stop=True)
                                ot = wkp.tile([128, TS], BF16, tag="ot",
                                              bufs=4, name="ot")
                                if mc % 2 == 0:
                                    nc.vector.tensor_copy(out=ot, in_=pc[:])
                                else:
                                    nc.scalar.copy(out=ot, in_=pc[:])
                                nc.sync.dma_start(out=outT_t[:, mc, tsl],
                                                  in_=ot)

    nc.finalize()
    return nc


# --------------------------------------------------------------------------
# Host-side input prep
# --------------------------------------------------------------------------
def _pack_lhsT(w):
    """[Dk, N] -> [N//128, 128, Dk//128, 128] contiguous lhsT tiles."""
    Dk, N = w.shape
    return np.ascontiguousarray(
        w.reshape(Dk // 128, 128, N // 128, 128).transpose(2, 1, 0, 3))


def prepare_weights(wq, bq, wk, bk, wv, bv, w_proj, b_proj, g1, be1, g2, be2,
                    w_fc, b_fc, w_cp, b_cp):
    """Fold LN affines + 1/sqrt(HD) + reshape heads; packed bf16 arrays."""
    bf = ml_dtypes.bfloat16
    H_, D_, HD_ = wq.shape
    qscale = 1.0 / np.sqrt(HD_)
    wq2 = wq.transpose(1, 0, 2).reshape(D_, H_ * HD_).astype(np.float64)
    wk2 = wk.transpose(1, 0, 2).reshape(D_, H_ * HD_).astype(np.float64)
    wv2 = wv.transpose(1, 0, 2).reshape(D_, H_ * HD_).astype(np.float64)
    g1 = g1.astype(np.float64); be1 = be1.astype(np.float64)
    g2 = g2.astype(np.float64); be2 = be2.astype(np.float64)
    w_fc64 = w_fc.astype(np.float64)
    arrs = {
        "wq": _pack_lhsT((qscale * g1[:, None] * wq2).astype(bf)),
        "wk": _pack_lhsT((g1[:, None] * wk2).astype(bf)),
        "wv": np.ascontiguousarray(
            (g1[:, None] * wv2).astype(bf)
            .reshape(-1, 128, wv2.shape[1]).transpose(1, 0, 2)),
        "wp": _pack_lhsT(w_proj.astype(bf)),
        "wf": _pack_lhsT((g2[:, None] * w_fc64).astype(bf)),
        "wc": np.ascontiguousarray(
            w_cp.astype(bf).reshape(-1, 128, w_cp.shape[1] // 128, 128)),
    }
    bias_arrs = {
        "bq": (bq.reshape(-1).astype(np.float64) + be1 @ wq2) * qscale,
        "bk": bk.reshape(-1).astype(np.float64) + be1 @ wk2,
        "bv": bv.reshape(-1).astype(np.float64) + be1 @ wv2,
        "bp": b_proj.astype(np.float64),
        "bf": b_fc.astype(np.float64) + be2 @ w_fc64,
        "bc": b_cp.astype(np.float64),
    }
    any_bias = bool(any(np.any(v != 0) for v in bias_arrs.values()))
    if any_bias:
        for k, v in bias_arrs.items():
            arrs[k] = v.astype(bf).reshape(1, -1)
    return arrs, any_bias


_NC_CACHE = {}


def kernel(**inputs):
    x = np.asarray(inputs["x"], np.float32)
    arrs, any_bias = prepare_weights(
        *(np.asarray(inputs[k]) for k in (
            "wq", "bq", "wk", "bk", "wv", "bv", "w_proj", "b_proj",
            "g1", "be1", "g2", "be2", "w_fc", "b_fc", "w_cp", "b_cp")))
    key = ("full", any_bias)
    if key not in _NC_CACHE:
        _NC_CACHE[key] = build_decoder_nc(with_bias=any_bias)
    nc = _NC_CACHE[key]

    in_maps = []
    for b in range(N_CORES):
        m = dict(arrs)
        m["xT"] = np.ascontiguousarray(x[b].T.astype(ml_dtypes.bfloat16))
        in_maps.append(m)

    from concourse.bass_utils import run_bass_kernel_spmd
    res = run_bass_kernel_spmd(nc, in_maps, list(range(N_CORES)))
    out = np.stack([res.results[i]["outT"].T.astype(np.float32)
                    for i in range(N_CORES)])
    return out



# revision 86
# speedup vs baseline: 1.0183x; 1.0183x over previous
"""Trainium2 Bass kernel for nn.DecoderBlock (pre-LN GPT block).

Shapes: B=8, T=1024, D=768, H=12, HD=64, F=3072.  Data-parallel: batch
element b runs on core b (no collectives needed).  All activations are
feature-major on chip ([D, T]: features on partitions, tokens free) so
chained matmuls need no transposes; attention scores are computed
transposed (S^T[t, q]) so softmax-weighted probabilities land directly in
the layout the P@V matmul needs.

Scheduling (the performance core of this kernel — ~1.6x over the naive
ordering of the same matmuls):
  * No GpSimd anywhere: partition broadcasts (LN apply, softmax
    denominator) are K=1 rank-1 PE matmuls into PSUM; causal masks are
    DVE multiplies on the exp output.  GpSimd semaphore+op latency
    (~1.3us per op) previously serialized the whole attention phase.
  * One ACT exp instruction per key-chunk covers BOTH heads of a
    128-partition pair via a two-bank PSUM score tile [128, 2, TQ]
    (halves ACT instruction-overhead; ACT is the attention-phase
    co-bottleneck with the PE).
  * P@V matmuls are software-pipelined two steps behind the score
    matmuls, with next-pair QK-projection matmuls split into 2-matmul
    micro-chunks pumped between steps, so the exp latency never idles
    the PE (PE idle gaps > ~3.4us re-throttle the PE clock to 1.2GHz —
    HAM — which was worth ~170us in the naive schedule).
  * V for ALL heads is computed upfront in 8 token-chunk units with the
    weight matrix as the moving operand (N=512/256 instead of N=128).
  * MLP is software-pipelined: ph(fc+1) matmuls are emitted before
    cp(fc), so each gelu runs under the next block of matmuls; final
    residual adds + output DMAs interleave into the last cp group.
  * LayerNorm: squares and PSUM evacuations on DVE; per-token stat
    chains run 128-lane in token-major layout (PE transposes); the
    apply uses rank-1 PE broadcasts.  All four ln_finish chains are
    generators whose PE ops pump between independent matmuls (stats(1)
    covers finish(0) at startup; V/QK units cover finish(1); the
    proj(1) units cover LN2's finish(0); pre-MLP fc units cover LN2's
    finish(1)), so their DVE serial chains never idle the PE.
  * LN2 runs entirely under the attention/proj tail: tci0 stats are
    last-pair filler, tci0's finish pumps into proj(1), tci1 mu AND
    sumsq stats interleave per proj(1) unit (separate PSUM banks so
    both groups stay open), and the first four fc units of the MLP run
    in the attention PSUM pools while finish(1) completes.
  * Several stat rows share one PSUM bank (matmul start=True clears
    only the bank's has_written bits, not its data), so the whole LN
    stat machinery needs just 3 banks across both layernorms.
  * Softmax normalization: the per-query denominator comes free as a
    65th ones-column in the P@V matmul; it is broadcast across
    partitions by a rank-1 PE matmul and reciprocated at full 128-lane
    DVE width after the broadcast (a [1,1024] single-lane recip is
    ~3x slower).
  * Host-side: LN affines and the 1/sqrt(HD) score scale are folded
    into the weights; weights are packed into DMA-contiguous lhsT
    tiles; matmuls run in bf16 with fp32 PSUM accumulation (fp8 was
    measured in simulation to breach the 2e-2 error budget: e4m3
    rounding alone contributes ~2.1% on the fc matmul).
  * Softmax max-subtraction is skipped: post-LN scores are O(5) so fp32
    exp cannot overflow.
  * The whole residual stream is bf16: x arrives bf16 from the host
    (kills the startup f32 DMA + on-chip cast; halves input DMA), X1 is
    bf16 (kills the LN2 scratch copies), the output leaves bf16 and is
    upcast on the host (halves output DMA, shortens the tail).
  * rstd = 1/sqrt(var+eps) runs entirely on DVE (bit-trick seed + 1
    Newton step, ~0.18% worst-case) instead of ACT Ln+Exp: each of the
    4 ln_finish calls previously triggered TWO ~1.3us ACT table loads
    on the critical path; now only the exp set (preloaded by a dummy at
    t=0) and the gelu set (preloaded right after proj(1)) are ever
    loaded, both off the PE critical path.
  * HAM warmup: ~100 junk matmuls on a memset tile run during the
    ~13us x-DMA wait so the LN1 stats chain starts at 2.4GHz instead
    of the cold 1.2GHz clock.
  * Softmax normalization of each (pair, qc) is deferred until the
    next score batch is in the PE queue, so its DVE work (den2 evac,
    recip, YT scale) overlaps scores instead of gating them.
  * Final residual adds are folded into the cp PSUM accumulation as
    identity matmuls; the output evacuations alternate DVE/ACT and the
    bf16 output DMA is half the size, shortening the post-last-matmul
    tail to ~6us.
"""

import numpy as np
import ml_dtypes

import concourse.bass as bass
import concourse.mybir as mybir
import concourse.tile as tile
from concourse import bacc

BF16 = mybir.dt.bfloat16
F32 = mybir.dt.float32
AF = mybir.ActivationFunctionType
OP = mybir.AluOpType

B, T, D, H = 8, 1024, 768, 12
HD = D // H
F = 4 * D
EPS = 1e-5
N_CORES = 8

KC = D // 128   # 6  contraction chunks over D
MC = D // 128   # 6  output-feature chunks over D
FC = F // 128   # 24 chunks over MLP hidden
NT = T // 128   # 8  key chunks
TS = 512        # token chunk (LN/proj/MLP)
TQ = 512        # query chunk
NQ = T // TQ    # 2
NJ = TS // 128  # 4
HPC = 2         # heads per 128-partition group
NPAIR = H // HPC  # 6
VS = HD + 1     # V columns per head incl ones-column (denominator row)


def build_decoder_nc(with_bias=False, eps=EPS, gelu_func=AF.Gelu_apprx_tanh):
    scale = 1.0  # 1/sqrt(HD) folded into wq host-side

    nc = bacc.Bacc()

    xT = nc.dram_tensor("xT", [D, T], BF16, kind="ExternalInput")
    wq_d = nc.dram_tensor("wq", [MC, 128, KC, 128], BF16, kind="ExternalInput")
    wk_d = nc.dram_tensor("wk", [MC, 128, KC, 128], BF16, kind="ExternalInput")
    wv_d = nc.dram_tensor("wv", [128, KC, D], BF16, kind="ExternalInput")
    wp_d = nc.dram_tensor("wp", [MC, 128, KC, 128], BF16, kind="ExternalInput")
    wf_d = nc.dram_tensor("wf", [FC, 128, KC, 128], BF16, kind="ExternalInput")
    wc_d = nc.dram_tensor("wc", [FC, 128, MC, 128], BF16, kind="ExternalInput")
    bias_d = {}
    if with_bias:
        for nm, width in (("bq", D), ("bk", D), ("bv", D), ("bp", D),
                          ("bf", F), ("bc", D)):
            bias_d[nm] = nc.dram_tensor(nm, [1, width], BF16,
                                        kind="ExternalInput")
    outT = nc.dram_tensor("outT", [D, T], BF16, kind="ExternalOutput")
    outT_t = outT[:].rearrange("(o p) t -> p o t", p=128)

    # ---- constants ----
    ones_bf = nc.inline_tensor(np.ones((1, T), ml_dtypes.bfloat16), "ones_bf")
    onescol_f = nc.inline_tensor(np.ones((128, 1), np.float32), "onescol_f")
    onescol_b = nc.inline_tensor(np.ones((128, 1), ml_dtypes.bfloat16),
                                 "onescol_b")
    onesrow_f = nc.inline_tensor(np.ones((1, 128), np.float32), "onesrow_f")
    onesrow_b = nc.inline_tensor(np.ones((1, 128), ml_dtypes.bfloat16),
                                 "onesrow_b")
    ident_b = nc.inline_tensor(np.eye(128, dtype=ml_dtypes.bfloat16),
                               "ident_b")
    # sel2[h, m] = 1 if m // 64 == h  (half-select broadcast)
    sel2_np = (np.arange(128)[None, :] // 64 ==
               np.arange(2)[:, None]).astype(np.float32)
    sel2_c = nc.inline_tensor(sel2_np, "sel2_c")
    # sel4[k, m] = 1 if m // 128 == k  (ln-stat row-select broadcast)
    sel4_np = (np.arange(512)[None, :] // 128 ==
               np.arange(4)[:, None]).astype(ml_dtypes.bfloat16)
    sel4_c = nc.inline_tensor(sel4_np, "sel4_c")
    ident_c = nc.inline_tensor(np.eye(128, dtype=np.float32), "ident_c")
    # multiplicative causal mask for transposed diagonal blocks: 1 if t <= q
    m_np = (np.arange(128)[:, None] <= np.arange(128)[None, :]).astype(
        ml_dtypes.bfloat16)
    masks_d = nc.inline_tensor(m_np, "masks")

    with tile.TileContext(nc) as tc:
        with (
            tc.tile_pool(name="persist", bufs=1) as pp,
            tc.tile_pool(name="wts", bufs=4) as wpool,
            tc.tile_pool(name="work", bufs=4) as wkp,
            tc.tile_pool(name="small", bufs=1) as sp,
        ):
            # ---- persistent SBUF ----
            ALN = pp.tile([128, KC, T], BF16, tag="ALN", name="ALN")
            QT = pp.tile([128, MC, T], BF16, tag="QT", name="QT")
            KT = pp.tile([128, MC, T], BF16, tag="KT", name="KT")
            Vt = pp.tile([128, NT, H * VS], BF16, tag="Vt", name="Vt")
            YT = pp.tile([128, KC, T], BF16, tag="YT", name="YT")
            X1 = pp.tile([128, KC, T], BF16, tag="X1", name="X1")
            A2 = pp.tile([128, KC, T], BF16, tag="A2", name="A2")
            wv_s = pp.tile([128, KC, D], BF16, tag="wv", name="wv_s")

            onescol_fs = pp.tile([128, 1], F32, tag="oc_f", name="onescol_fs")
            onescol_bs = pp.tile([128, 1], BF16, tag="oc_b", name="onescol_bs")
            onesrow_fs = pp.tile([1, 128], F32, tag="or_f", name="onesrow_fs")
            onesrow_bs = pp.tile([1, 128], BF16, tag="or_b", name="onesrow_bs")
            ident_bs = pp.tile([128, 128], BF16, tag="id_b", name="ident_bs")
            sel2_s = pp.tile([2, 128], F32, tag="sel2", name="sel2_s")
            sel4_s = pp.tile([4, 512], BF16, tag="sel4", name="sel4_s")
            ident_s = pp.tile([128, 128], F32, tag="ident", name="ident_s")
            masks_s = pp.tile([128, 128], BF16, tag="masks", name="masks_s")
            nc.sync.dma_start(out=onescol_fs, in_=onescol_f[:])
            nc.sync.dma_start(out=onescol_bs, in_=onescol_b[:])
            nc.sync.dma_start(out=onesrow_fs, in_=onesrow_f[:])
            nc.sync.dma_start(out=onesrow_bs, in_=onesrow_b[:])
            nc.sync.dma_start(out=ident_bs, in_=ident_b[:])
            nc.sync.dma_start(out=sel2_s, in_=sel2_c[:])
            nc.sync.dma_start(out=sel4_s, in_=sel4_c[:])
            nc.sync.dma_start(out=ident_s, in_=ident_c[:])
            nc.sync.dma_start(out=masks_s, in_=masks_d[:])
            onesb_s = None
            biases = {}
            if with_bias:
                onesb_s = pp.tile([1, T], BF16, tag="onesb", name="onesb_s")
                nc.sync.dma_start(out=onesb_s, in_=ones_bf[:])
                for nm, dten in bias_d.items():
                    bt = pp.tile(list(dten.shape), BF16, tag=nm,
                                 name=f"{nm}_s")
                    nc.sync.dma_start(out=bt, in_=dten[:])
                    biases[nm] = bt

            def bias_mm(psum, bias_t, msl, tsl):
                if bias_t is None:
                    return True
                nc.tensor.matmul(psum, bias_t[0:1, msl], onesb_s[0:1, tsl],
                                 start=True, stop=False)
                return False

            # ================= attention-phase PSUM pools =================
            with (
                tc.tile_pool(name="psA", bufs=2, space="PSUM") as psA,  # 4 banks
                tc.tile_pool(name="psB", bufs=1, space="PSUM") as psB,  # 2 banks
                tc.tile_pool(name="psC", bufs=2, space="PSUM") as psC,  # 2 banks
            ):
                # ---- load x^T (bf16 from host) BEFORE the V weights: the
                # LN1 stats chain gates everything, and per-kc full-T chunks
                # keep 2KB DMA lines (a tci-split halves line size and DMA
                # efficiency) ----
                xT_t = xT[:].rearrange("(o p) t -> p o t", p=128)
                for kc in range(KC):
                    nc.sync.dma_start(out=A2[:, kc, :], in_=xT_t[:, kc, :])
                nc.sync.dma_start(out=wv_s, in_=wv_d[:])

                # ---- LayerNorm (stats + apply); bf16 src, bf16 dst ----
                # row_mu / row_sq are [1, TS] PSUM APs.  A matmul start=True
                # zeroes the has_written bits of the whole 2KB bank, but not
                # the data, so many stat rows can share a bank as long as
                # their accumulation groups are strictly sequential.
                def ln_stats(sb, tci, row_mu, row_sq, sq, sqsl=None):
                    tsl = slice(tci * TS, (tci + 1) * TS)
                    for kc in range(KC):
                        nc.tensor.matmul(row_mu, onescol_bs[:],
                                         sb[:, kc, tsl],
                                         start=(kc == 0), stop=(kc == KC - 1))
                    for kc in range(KC):
                        nc.tensor.matmul(row_sq, onescol_bs[:],
                                         sq[:, kc, sqsl or slice(None)],
                                         start=(kc == 0), stop=(kc == KC - 1))

                def ln_finish_gen(src, dst, tci, row_mu, row_sq,
                                  act_evac=True, prep_pool=None,
                                  coarse_apply=False):
                    tsl = slice(tci * TS, (tci + 1) * TS)
                    srow_mu = sp.tile([1, TS], F32, tag="srow_mu",
                                      name="srow_mu")
                    srow_sq = sp.tile([1, TS], F32, tag="srow_sq",
                                      name="srow_sq")
                    # evacuate the two stat rows on different engines so they
                    # run concurrently (GpSimd cannot read PSUM); near the
                    # attention->MLP seam ACT is backlogged with exp/gelu, so
                    # those call sites use DVE for both
                    if act_evac:
                        nc.scalar.copy(out=srow_mu, in_=row_mu)
                    else:
                        nc.vector.tensor_copy(out=srow_mu, in_=row_mu)
                    nc.vector.tensor_copy(out=srow_sq, in_=row_sq)
                    yield
                    # token-major [128, NJ, 2] via PE transposes
                    ptk = psC.tile([128, NJ, 2], F32, tag="psC", name="ptk")
                    for jj in range(NJ):
                        jsl = slice(jj * 128, (jj + 1) * 128)
                        nc.tensor.transpose(
                            ptk[:, jj, 0:1], srow_mu[:, jsl],
                            ident_s[0:1, 0:1])
                        nc.tensor.transpose(
                            ptk[:, jj, 1:2], srow_sq[:, jsl],
                            ident_s[0:1, 0:1])
                        yield
                    stok = sp.tile([128, NJ, 2], F32, tag="stok", name="stok")
                    nc.vector.tensor_copy(out=stok, in_=ptk)
                    nc.vector.tensor_scalar_mul(stok, stok, 1.0 / D)
                    mu = stok[:, :, 0]
                    m2 = stok[:, :, 1]
                    var_t = sp.tile([128, NJ], F32, tag="var_t", name="var_t")
                    nc.vector.tensor_tensor(var_t, mu, mu, OP.mult)
                    nc.vector.tensor_tensor(var_t, m2, var_t, OP.subtract)
                    nc.vector.tensor_scalar_add(var_t, var_t, eps)
                    # st2[:,0,:]=rstd  st2[:,1,:]=-mu*rstd
                    # rstd = 1/sqrt(var+eps) entirely on DVE: bit-trick seed
                    # + 2 Newton steps.  No ACT Ln/Exp -> no ~1.3us ACT table
                    # loads on the LN critical path (2 per finish before).
                    st2 = sp.tile([128, 2, NJ], F32, tag="st2", name="st2")
                    y = st2[:, 0, :]
                    nr = sp.tile([128, NJ], F32, tag="nr_tmp", name="nr_tmp")
                    nc.vector.tensor_scalar(
                        y.bitcast(mybir.dt.int32),
                        var_t[:, :].bitcast(mybir.dt.int32),
                        1, None, OP.logical_shift_right)
                    # magic - t  ==  ~t + (magic + 1)  (two's complement);
                    # bitwise and arith ops can't share one instruction
                    nc.vector.tensor_scalar(
                        y.bitcast(mybir.dt.int32), y.bitcast(mybir.dt.int32),
                        0, None, OP.bitwise_not)
                    nc.vector.tensor_scalar(
                        y.bitcast(mybir.dt.int32), y.bitcast(mybir.dt.int32),
                        0x5f375a86 + 1, None, OP.add)
                    # one Newton step: <=0.18% rstd error, invisible next to
                    # the bf16 noise floor, and ~0.7us less serial DVE per
                    # ln_finish chain
                    for _ in range(1):
                        nc.vector.tensor_tensor(nr, y, y, OP.mult)
                        nc.vector.tensor_tensor(nr, nr, var_t, OP.mult)
                        nc.vector.tensor_scalar(nr, nr, -0.5, 1.5,
                                                OP.mult, OP.add)
                        nc.vector.tensor_tensor(y, y, nr, OP.mult)
                    nc.vector.tensor_tensor(st2[:, 1, :], mu, y, OP.mult)
                    nc.vector.tensor_scalar_mul(st2[:, 1, :], st2[:, 1, :],
                                                -1.0)
                    yield
                    # back to row layout per stat (all base-partition-0 APs)
                    prow = psC.tile([NJ, 2, 128], F32, tag="psC", name="prow")
                    nc.tensor.transpose(prow[:, 0, :], st2[:, 0, :],
                                        ident_s[:])
                    nc.tensor.transpose(prow[:, 1, :], st2[:, 1, :],
                                        ident_s[:])
                    yield
                    rows_sb = sp.tile([NJ, 2, 128], BF16, tag="rows_sb",
                                      name="rows_sb")
                    nc.vector.tensor_copy(out=rows_sb, in_=prow)
                    yield
                    # broadcast each 128-token stat row across partitions via
                    # a K=4 row-select matmul -- replaces a ~1.7us SBUF
                    # gather DMA that used to sit on this chain
                    pool = prep_pool if prep_pool is not None else psA
                    prep = pool.tile([128, 2, TS], F32, tag=pool.name,
                                     name="prep")
                    for jj in range(NJ):
                        jsl = slice(jj * 128, (jj + 1) * 128)
                        for sti in range(2):
                            nc.tensor.matmul(
                                prep[:, sti, jsl], sel4_s[:, jsl],
                                rows_sb[:, sti, :],
                                start=True, stop=True)
                    yield
                    tmp = wkp.tile([128, KC, TS], BF16, tag="lntmp", bufs=1,
                                   name="lntmp")
                    if coarse_apply:
                        # one whole-tci mult + add: consumers need the full
                        # half anyway, and 2 big DVE ops retire ~2x sooner
                        # than 8 per-jj ones (less per-op overhead + queue)
                        nc.vector.tensor_tensor(
                            tmp, src[:, :, tsl],
                            prep[:, 0:1, :].to_broadcast((128, KC, TS)),
                            OP.mult)
                        nc.vector.tensor_tensor(
                            dst[:, :, tsl], tmp,
                            prep[:, 1:2, :].to_broadcast((128, KC, TS)),
                            OP.add)
                        return
                    for jj in range(NJ):
                        jsl = slice(jj * 128, (jj + 1) * 128)
                        jtl = slice(tci * TS + jj * 128,
                                    tci * TS + (jj + 1) * 128)
                        # jj0 in two kc-halves: the first V/QK consumer can
                        # start on kc 0-2 ~1us before the full chunk lands
                        khs = ([slice(0, KC // 2), slice(KC // 2, KC)]
                               if jj == 0 else [slice(0, KC)])
                        for kh in khs:
                            nkc = kh.stop - kh.start
                            nc.vector.tensor_tensor(
                                tmp[:, kh, jsl], src[:, kh, jtl],
                                prep[:, 0:1, jsl].to_broadcast(
                                    (128, nkc, 128)), OP.mult)
                            nc.vector.tensor_tensor(
                                dst[:, kh, jtl], tmp[:, kh, jsl],
                                prep[:, 1:2, jsl].to_broadcast(
                                    (128, nkc, 128)), OP.add)

                def ln_finish(src, dst, tci, row_mu, row_sq, act_evac=True):
                    for _ in ln_finish_gen(src, dst, tci, row_mu, row_sq,
                                           act_evac):
                        pass

                # ---- V units (one per 128-token chunk, all heads) ----
                Vt4 = Vt.rearrange("p t (h c) -> p t h c", c=VS)
                nc.vector.memset(Vt4[:, :, :, HD:HD + 1], 1.0)

                def v_unit(tch):
                    t128 = slice(tch * 128, (tch + 1) * 128)
                    pvv = psA.tile([128, 2, TS], F32, tag="psA", name="pvv")
                    st0 = True
                    st1 = True
                    if with_bias:
                        nc.tensor.matmul(pvv[:, 0, :], onesb_s[0:1, 0:128],
                                         biases["bv"][0:1, 0:512],
                                         start=True, stop=False)
                        nc.tensor.matmul(pvv[:, 1, 0:256],
                                         onesb_s[0:1, 0:128],
                                         biases["bv"][0:1, 512:768],
                                         start=True, stop=False)
                        st0 = st1 = False
                    for kc in range(KC):
                        nc.tensor.matmul(
                            pvv[:, 0, :], ALN[:, kc, t128],
                            wv_s[:, kc, 0:512],
                            start=st0 and (kc == 0), stop=(kc == KC - 1))
                        nc.tensor.matmul(
                            pvv[:, 1, 0:256], ALN[:, kc, t128],
                            wv_s[:, kc, 512:768],
                            start=st1 and (kc == 0), stop=(kc == KC - 1))
                    # split the evacuation across ACT and DVE
                    nc.scalar.copy(
                        out=Vt4[:, tch, 0:8, 0:HD],
                        in_=pvv[:, 0, :].rearrange("p (h c) -> p h c", c=HD))
                    nc.vector.tensor_copy(
                        out=Vt4[:, tch, 8:12, 0:HD],
                        in_=pvv[:, 1, 0:256].rearrange("p (h c) -> p h c",
                                                       c=HD))

                # ---- QK unit generators (pair mc); yield per 2 matmuls ----
                def make_qk_gens(mc, tcis=(0, 1), on_act=False):
                    msl = slice(mc * 128, (mc + 1) * 128)
                    gens = []
                    wts = []
                    for nm, wten, dstT in (("bq", wq_d, QT), ("bk", wk_d, KT)):
                        wt = wpool.tile([128, KC, 128], BF16, tag="w_qk",
                                        bufs=4, name="wt_qk")
                        nc.sync.dma_start(out=wt, in_=wten[mc])
                        wts.append((nm, wt, dstT))
                    for tci in tcis:
                        for nm, wt, dstT in wts:
                            def qk_gen(nm=nm, wt=wt, dstT=dstT, tci=tci,
                                       on_act=on_act):
                                tsl = slice(tci * TS, (tci + 1) * TS)
                                pq = psC.tile([128, TS], F32, tag="psC",
                                              name="pq")
                                st = bias_mm(pq, biases.get(nm), msl, tsl)
                                for kc in range(KC):
                                    nc.tensor.matmul(
                                        pq, wt[:, kc, :], ALN[:, kc, tsl],
                                        start=st and (kc == 0),
                                        stop=(kc == KC - 1))
                                    if kc % 2 == 1 and kc < KC - 1:
                                        yield
                                if on_act:
                                    nc.scalar.copy(out=dstT[:, mc, tsl],
                                                   in_=pq)
                                else:
                                    nc.vector.tensor_copy(
                                        out=dstT[:, mc, tsl], in_=pq)
                            gens.append(qk_gen())
                    return gens

                # ---- proj unit generators (attn out-proj + residual) ----
                def make_proj_gens(tci):
                    tsl = slice(tci * TS, (tci + 1) * TS)
                    gens = []
                    for mc in range(MC):
                        wt = wpool.tile([128, KC, 128], BF16, tag="w_p",
                                        bufs=4, name="wt_p")
                        nc.sync.dma_start(out=wt, in_=wp_d[mc])

                        def proj_gen(mc=mc, wt=wt):
                            msl = slice(mc * 128, (mc + 1) * 128)
                            po = psC.tile([128, TS], F32, tag="psC", name="po")
                            st = bias_mm(po, biases.get("bp"), msl, tsl)
                            for kc in range(KC):
                                nc.tensor.matmul(
                                    po, wt[:, kc, :], YT[:, kc, tsl],
                                    start=st and (kc == 0), stop=False)
                                if kc % 2 == 1 and kc < KC - 1:
                                    yield
                            # residual folded into the PE accumulation
                            # (identity @ X); the X1 evacuation then runs on
                            # the idle ACT instead of the congested DVE
                            nc.tensor.matmul(
                                po, ident_bs[:], A2[:, mc, tsl],
                                start=False, stop=True)
                            nc.scalar.copy(out=X1[:, mc, tsl], in_=po)
                        gens.append(proj_gen(mc, wt))
                    return gens

                def pump(gens, n):
                    """Advance the generator queue by n yield-chunks."""
                    while n > 0 and gens:
                        try:
                            next(gens[0])
                        except StopIteration:
                            gens.pop(0)
                            continue
                        n -= 1

                def drain(gens):
                    for g in gens:
                        for _ in g:
                            pass
                    gens.clear()

                # ---- startup: LN1 overlapped with V and pair-0 QK ----
                # HAM warmup: the PE clock starts throttled at 1.2GHz and
                # only unthrottles after ~3.4us of sustained activity.  The
                # x DMA takes ~13us to land, so burn that wait on junk
                # matmuls (memset source: no DMA dependency) to both warm
                # the clock and keep it warm until the stats arrive.
                wu_src = sp.tile([128, 128], BF16, tag="wu_src", name="wu_src")
                nc.vector.memset(wu_src, 1.0)
                # trigger the exp table-set load NOW: it covers Copy too, so
                # ACT never loads a table again until the MLP gelu
                wu_act = sp.tile([1, 128], F32, tag="wu_act", name="wu_act")
                nc.scalar.activation(out=wu_act, in_=wu_src[0:1, :],
                                     func=AF.Exp)
                junk = psA.tile([128, 2, TQ], F32, tag="psA", name="junk")
                for _ in range(135):
                    nc.tensor.matmul(junk[:, 0, 0:128], wu_src[:],
                                     wu_src[:], start=True, stop=True)
                # x squares for BOTH token halves upfront: DVE computes them
                # chunk-by-chunk as the x DMA lands, while the PE is still on
                # warmup junk -- so the DVE is free later when the ln_finish
                # chains need it
                sq_all = wkp.tile([128, KC, T], BF16, tag="sq_all", bufs=1,
                                  name="sq_all")
                for kc in range(KC):
                    nc.vector.tensor_tensor(sq_all[:, kc, :], A2[:, kc, :],
                                            A2[:, kc, :], OP.mult)
                pstat1 = psB.tile([128, 2, TS], F32, tag="psB", name="pstat1")
                ln_stats(A2, 0, pstat1[0:1, 0, :], pstat1[0:1, 1, :],
                         sq_all, slice(0, TS))
                # finish(0) must fully drain before any ALN(tci0) consumer
                # matmul is emitted (the in-order PE queue would deadlock);
                # the stats(1) matmuls are pumped in as its latency cover.
                fin0 = [ln_finish_gen(A2, ALN, 0, pstat1[0:1, 0, :],
                                      pstat1[0:1, 1, :])]
                pump(fin0, 1)  # stat-row evacuations start now
                ln_stats(A2, 1, pstat1[64:65, 0, :], pstat1[64:65, 1, :],
                         sq_all, slice(TS, T))
                drain(fin0)
                # fin1's prep goes in psB (pstat1's slot, dead after its own
                # evacuations): in psA it would join the pvv rotation and
                # stall v_unit(4) on the whole LN1 apply(1)
                qk0_a = make_qk_gens(0, tcis=(0,), on_act=True)
                qk0_a.append(ln_finish_gen(A2, ALN, 1, pstat1[64:65, 0, :],
                                           pstat1[64:65, 1, :],
                                           prep_pool=psB))
                for tch in range(4):
                    v_unit(tch)
                    pump(qk0_a, 6)
                drain(qk0_a)
                qk0_b = make_qk_gens(0, tcis=(1,), on_act=True)
                for tch in range(4, NT):
                    v_unit(tch)
                    pump(qk0_b, 3)
                drain(qk0_b)
                # LN2(tci0) stats + finish as last-pair filler generators:
                # pumped after proj(0) drains, so apply(0) runs on DVE while
                # attention finishes and MLP can start right after proj(1).
                # pstat0 is allocated lazily (at first pump) so the psC
                # rotation during earlier pairs cannot clobber it.
                pstat0_box = []

                def stats0_gen():
                    pstat0_box.append(
                        psC.tile([128, TS], F32, tag="psC", name="pstat0"))
                    pstat0 = pstat0_box[0]
                    tsl0 = slice(0, TS)
                    sq = wkp.tile([128, KC, TS], BF16, tag="sq", bufs=1,
                                  name="sq")
                    for kc in range(KC):
                        nc.vector.tensor_tensor(
                            sq[:, kc, :], X1[:, kc, tsl0], X1[:, kc, tsl0],
                            OP.mult)
                    for kc in range(KC):
                        nc.tensor.matmul(pstat0[0:1, :], onescol_bs[:],
                                         X1[:, kc, tsl0],
                                         start=(kc == 0), stop=(kc == KC - 1))
                        if kc % 2 == 1 and kc < KC - 1:
                            yield
                    for kc in range(KC):
                        nc.tensor.matmul(pstat0[32:33, :], onescol_bs[:],
                                         sq[:, kc, :],
                                         start=(kc == 0), stop=(kc == KC - 1))
                        if kc % 2 == 1:
                            yield

                # finish(0)-of-LN2 starts inside the last-pair filler: its
                # stat-row evacuations then sit in the DVE queue AHEAD of the
                # last normalize/square batch instead of behind it
                fin0_box = []

                def fin0_start():
                    p0 = pstat0_box[0]
                    g = ln_finish_gen(X1, A2, 0, p0[0:1, :], p0[32:33, :],
                                      act_evac=False, coarse_apply=True)
                    fin0_box.append(g)
                    next(g)
                    yield

                norm_pend = []
                for mc in range(NPAIR):
                    last = mc + 1 >= NPAIR
                    filler = (make_proj_gens(0) if last
                              else make_qk_gens(mc + 1))
                    if last:
                        filler.append(stats0_gen())
                        filler.append(fin0_start())
                    for qc in range(NQ):
                        qsl = slice(qc * TQ, (qc + 1) * TQ)
                        ntch = (qc + 1) * (TQ // 128)
                        py = psB.tile([128, 2, TQ], F32, tag="psB", name="py")
                        pv_pend = []

                        def emit_pv(tch, pexp, rq, ntch=ntch, py=py, mc=mc):
                            for half in range(HPC):
                                h = mc * HPC + half
                                nc.tensor.matmul(
                                    py[0:VS, half, rq],
                                    Vt[:, tch, h * VS:(h + 1) * VS],
                                    pexp[:, half, rq],
                                    start=(tch == 0), stop=(tch == ntch - 1))

                        for tch in range(ntch):
                            t128 = slice(tch * 128, (tch + 1) * 128)
                            diag0 = qc * (TQ // 128)
                            dq = max(0, tch - diag0) * 128
                            rq = slice(dq, TQ)
                            qslr = slice(qc * TQ + dq, (qc + 1) * TQ)
                            psc = psA.tile([128, 2, TQ], F32, tag="psA",
                                           name="psc")
                            for half in range(HPC):
                                hsl = slice(half * HD, (half + 1) * HD)
                                nc.tensor.matmul(
                                    psc[:, half, rq], KT[hsl, mc, t128],
                                    QT[hsl, mc, qslr], start=True, stop=True)
                            # previous (pair, qc)'s softmax-normalize runs
                            # here, after this qc's first scores are already
                            # in the PE queue: its DVE work overlaps the PE
                            # instead of gating it at the qc boundary
                            if tch == 1 and norm_pend:
                                norm_pend.pop(0)()
                            pexp = wkp.tile([128, 2, TQ], BF16, tag="pexp",
                                            bufs=5, name="pexp")
                            nc.scalar.activation(out=pexp[:, :, rq],
                                                 in_=psc[:, :, rq],
                                                 func=AF.Exp)
                            if tch >= diag0:
                                nc.vector.tensor_tensor(
                                    pexp[:, :, dq:dq + 128],
                                    pexp[:, :, dq:dq + 128],
                                    masks_s[:, None, :].to_broadcast(
                                        (128, 2, 128)), OP.mult)
                            pv_pend.append((tch, pexp, rq))
                            # filler micro-chunks BEFORE the PV: the PE queue
                            # is in-order, so a PV stalled on its exp must
                            # not trap independent filler matmuls behind it
                            if qc == NQ - 1 or not last:
                                pump(filler, 4 if last else 1)
                            # software pipeline: PV three steps behind
                            # scores so the exp latency never stalls the PE
                            if len(pv_pend) > 3:
                                emit_pv(*pv_pend.pop(0))
                        if qc == NQ - 1 or not last:
                            pump(filler, 2)
                        while pv_pend:
                            emit_pv(*pv_pend.pop(0))

                        # softmax normalization (no GpSimd): both halves'
                        # denominators side by side in one base-0 row
                        def normalize(py=py, qsl=qsl, mc=mc):
                            den2 = sp.tile([1, 2 * TQ], BF16, tag="den2",
                                           name="den2")
                            for half in range(HPC):
                                nc.vector.tensor_copy(
                                    out=den2[:, half * TQ:(half + 1) * TQ],
                                    in_=py[HD:HD + 1, half, :])
                            prep1 = psC.tile([128, TS], F32, tag="psC",
                                             name="prep1")
                            for half in range(HPC):
                                nc.tensor.matmul(
                                    prep1[half * HD:(half + 1) * HD, :],
                                    onesrow_bs[0:1, 0:HD],
                                    den2[:, half * TQ:(half + 1) * TQ],
                                    start=True, stop=True)
                            # reciprocal AFTER the broadcast: 128-lane DVE op
                            # instead of a slow single-partition recip
                            prep_sb = wkp.tile([128, TS], F32, tag="prep_sb",
                                               bufs=2, name="prep_sb")
                            nc.vector.reciprocal_approx_fast(out=prep_sb,
                                                             in_=prep1)
                            for half in range(HPC):
                                hsl = slice(half * HD, (half + 1) * HD)
                                nc.vector.tensor_tensor(
                                    YT[hsl, mc, qsl], py[0:HD, half, :],
                                    prep_sb[hsl, :], OP.mult)

                        if last and qc == 0:
                            # proj(0) filler reads YT(tci0) this pair's qc=1:
                            # cannot defer
                            normalize()
                        else:
                            norm_pend.append(normalize)
                    drain(filler)
                while norm_pend:
                    norm_pend.pop(0)()

                # Prefetch the first MLP weight tiles now so their DMAs don't
                # queue behind the LN2 rows DMAs right when the MLP starts.
                pre_wts = []
                for fc in range(4):
                    wt = wpool.tile([128, KC, 128], BF16, tag="w_f",
                                    bufs=4, name="wt_f")
                    nc.sync.dma_start(out=wt, in_=wf_d[fc])
                    wtc = wpool.tile([128, MC, 128], BF16, tag="w_c",
                                     bufs=4, name="wt_c")
                    nc.sync.dma_start(out=wtc, in_=wc_d[fc])
                    pre_wts.append((wt, wtc))
                # LN2(tci0) stats ran as last-pair filler; finish(0) here so
                # apply(0) runs on DVE underneath proj(1).  LN2(tci1) mu AND
                # sumsq both interleave with the proj(1) units -- they
                # accumulate in different PSUM banks so both groups can stay
                # open across the loop.
                # emit finish(0)'s transposes + DVE-chain now, but leave its
                # later PE ops (prow/preps/apply) to pump between the proj(1)
                # units: the DVE chain then runs UNDER the proj matmuls
                # instead of idling the PE
                pump(fin0_box, 5)
                pstL2 = psB.tile([128, 2, TS], F32, tag="psB", name="pstL2")
                proj1 = make_proj_gens(1)
                tsl1 = slice(TS, 2 * TS)
                sq1 = wkp.tile([128, KC, TS], BF16, tag="sq1", bufs=1,
                               name="sq1")
                for mc, g in enumerate(proj1):
                    for _ in g:
                        pass
                    nc.vector.tensor_tensor(sq1[:, mc, :], X1[:, mc, tsl1],
                                            X1[:, mc, tsl1], OP.mult)
                    nc.tensor.matmul(pstL2[0:1, 0, :], onescol_bs[:],
                                     X1[:, mc, tsl1],
                                     start=(mc == 0), stop=(mc == KC - 1))
                    nc.tensor.matmul(pstL2[0:1, 1, :], onescol_bs[:],
                                     sq1[:, mc, :],
                                     start=(mc == 0), stop=(mc == KC - 1))
                    if mc >= 2:
                        pump(fin0_box, 2)
                drain(fin0_box)
                # trigger the gelu table-set load now: the proj X1
                # evacuations are done with ACT and the first real gelu
                # (pre-MLP units below) then hits a warm table.  Any earlier
                # and the 1.3us load delays a proj evacuation -> PE stall.
                nc.scalar.activation(out=wu_act, in_=wu_src[0:1, :],
                                     func=gelu_func)
                # Pre-MLP: the first two fc(qc=0) units run here in the
                # attention PSUM pools (they only need apply(0), done under
                # proj(1)), with finish(1) pumped between them, so the PE
                # never drains while LN2(tci1) finishes on DVE.
                fin1 = [ln_finish_gen(X1, A2, 1,
                                      pstL2[0:1, 0, :], pstL2[0:1, 1, :],
                                      act_evac=False, coarse_apply=True)]
                pump(fin1, 1)  # stat-row evacuations start now
                pre_hgel = []
                tsl0 = slice(0, TS)
                for fc in range(4):
                    wt, _wtc = pre_wts[fc]
                    ph = psA.tile([128, 2, TQ], F32, tag="psA",
                                  name="ph_pre")
                    st = bias_mm(ph[:, 0, 0:TS], biases.get("bf"),
                                 slice(fc * 128, (fc + 1) * 128), tsl0)
                    for kc in range(KC):
                        nc.tensor.matmul(
                            ph[:, 0, 0:TS], wt[:, kc, :], A2[:, kc, tsl0],
                            start=st and (kc == 0), stop=(kc == KC - 1))
                    hgel = wkp.tile([128, TS], BF16, tag="hgel", bufs=6,
                                    name="hgel")
                    nc.scalar.activation(out=hgel, in_=ph[:, 0, 0:TS],
                                         func=gelu_func)
                    pre_hgel.append(hgel)
                # both pre-MLP units are emitted BEFORE the rest of
                # finish(1): its PE ops wait on the DVE chain, and the fc
                # matmuls are the cover
                drain(fin1)

            # ================= MLP phase (new PSUM pools) =================
            with (
                tc.tile_pool(name="psPC", bufs=6, space="PSUM") as psPC,
                tc.tile_pool(name="psPH", bufs=2, space="PSUM") as psPH,
            ):
                for qc in range(T // TS):
                    tsl = slice(qc * TS, (qc + 1) * TS)
                    pcs = []
                    for mc in range(MC):
                        pc = psPC.tile([128, TS], F32, tag="psPC",
                                       name=f"pc{mc}")
                        st = bias_mm(pc, biases.get("bc"),
                                     slice(mc * 128, (mc + 1) * 128), tsl)
                        pcs.append((pc, st))
                    cp_pend = []
                    if qc == 0:
                        for fc in range(4):
                            cp_pend.append((fc, pre_hgel[fc], pre_wts[fc][1]))

                    def emit_cp(fc, hgel, wtc, pcs=pcs):
                        for mc in range(MC):
                            pc, st = pcs[mc]
                            nc.tensor.matmul(
                                pc, wtc[:, mc, :], hgel,
                                start=st and (fc == 0), stop=(fc == FC - 1))

                    for fc in range(len(cp_pend), FC):
                        fsl = slice(fc * 128, (fc + 1) * 128)
                        wt = wpool.tile([128, KC, 128], BF16, tag="w_f",
                                        bufs=4, name="wt_f")
                        nc.sync.dma_start(out=wt, in_=wf_d[fc])
                        wtc = wpool.tile([128, MC, 128], BF16, tag="w_c",
                                         bufs=4, name="wt_c")
                        nc.sync.dma_start(out=wtc, in_=wc_d[fc])
                        ph = psPH.tile([128, TS], F32, tag="psPH", name="ph")
                        st = bias_mm(ph, biases.get("bf"), fsl, tsl)
                        for kc in range(KC):
                            nc.tensor.matmul(
                                ph, wt[:, kc, :], A2[:, kc, tsl],
                                start=st and (kc == 0), stop=(kc == KC - 1))
                        hgel = wkp.tile([128, TS], BF16, tag="hgel", bufs=6,
                                        name="hgel")
                        nc.scalar.activation(out=hgel, in_=ph, func=gelu_func)
                        cp_pend.append((fc, hgel, wtc))
                        # deep pipeline: the first cp write of each qc lands
                        # on PSUM banks whose previous reader (LN2 apply /
                        # the other qc's residual adds, both on DVE) may
                        # still be draining
                        if len(cp_pend) > 4:
                            emit_cp(*cp_pend.pop(0))
                    while cp_pend:
                        fc_l, hgel_l, wtc_l = cp_pend.pop(0)
                        for mc in range(MC):
                            pc, st = pcs[mc]
                            nc.tensor.matmul(
                                pc, wtc_l[:, mc, :], hgel_l,
                                start=st and (fc_l == 0), stop=False)
                            if fc_l == FC - 1:
                                # residual add folded into the PE
                                # accumulation; the evacuation is then a
                                # plain copy that alternates ACT/DVE so the
                                # output tail drains on two engines
                                nc.tensor.matmul(
                                    pc, ident_bs[:], X1[:, mc, tsl],
                                    start=False, stop=True)
                                ot = wkp.tile([128, TS], BF16, tag="ot",
                                              bufs=4, name="ot")
                                if mc % 2 == 0:
                                    nc.vector.tensor_copy(out=ot, in_=pc[:])
                                else:
                                    nc.scalar.copy(out=ot, in_=pc[:])
                                nc.sync.dma_start(out=outT_t[:, mc, tsl],
                                                  in_=ot)

    nc.finalize()
    return nc


# --------------------------------------------------------------------------
# Host-side input prep
# --------------------------------------------------------------------------
def _pack_lhsT(w):
    """[Dk, N] -> [N//128, 128, Dk//128, 128] contiguous lhsT tiles."""
    Dk, N = w.shape
    return np.ascontiguousarray(
        w.reshape(Dk // 128, 128, N // 128, 128).transpose(2, 1, 0, 3))


def prepare_weights(wq, bq, wk, bk, wv, bv, w_proj, b_proj, g1, be1, g2, be2,
                    w_fc, b_fc, w_cp, b_cp):
    """Fold LN affines + 1/sqrt(HD) + reshape heads; packed bf16 arrays."""
    bf = ml_dtypes.bfloat16
    H_, D_, HD_ = wq.shape
    qscale = 1.0 / np.sqrt(HD_)
    wq2 = wq.transpose(1, 0, 2).reshape(D_, H_ * HD_).astype(np.float64)
    wk2 = wk.transpose(1, 0, 2).reshape(D_, H_ * HD_).astype(np.float64)
    wv2 = wv.transpose(1, 0, 2).reshape(D_, H_ * HD_).astype(np.float64)
    g1 = g1.astype(np.float64); be1 = be1.astype(np.float64)
    g2 = g2.astype(np.float64); be2 = be2.astype(np.float64)
    w_fc64 = w_fc.astype(np.float64)
    arrs = {
        "wq": _pack_lhsT((qscale * g1[:, None] * wq2).astype(bf)),
        "wk": _pack_lhsT((g1[:, None] * wk2).astype(bf)),
        "wv": np.ascontiguousarray(
            (g1[:, None] * wv2).astype(bf)
            .reshape(-1, 128, wv2.shape[1]).transpose(1, 0, 2)),
        "wp": _pack_lhsT(w_proj.astype(bf)),
        "wf": _pack_lhsT((g2[:, None] * w_fc64).astype(bf)),
        "wc": np.ascontiguousarray(
            w_cp.astype(bf).reshape(-1, 128, w_cp.shape[1] // 128, 128)),
    }
    bias_arrs = {
        "bq": (bq.reshape(-1).astype(np.float64) + be1 @ wq2) * qscale,
        "bk": bk.reshape(-1).astype(np.float64) + be1 @ wk2,
        "bv": bv.reshape(-1).astype(np.float64) + be1 @ wv2,
        "bp": b_proj.astype(np.float64),
        "bf": b_fc.astype(np.float64) + be2 @ w_fc64,
        "bc": b_cp.astype(np.float64),
    }
    any_bias = bool(any(np.any(v != 0) for v in bias_arrs.values()))
    if any_bias:
        for k, v in bias_arrs.items():
            arrs[k] = v.astype(bf).reshape(1, -1)
    return arrs, any_bias


_NC_CACHE = {}


def kernel(**inputs):
    x = np.asarray(inputs["x"], np.float32)
    arrs, any_bias = prepare_weights(
        *(np.asarray(inputs[k]) for k in (
            "wq", "bq", "wk", "bk", "wv", "bv", "w_proj", "b_proj",
            "g1", "be1", "g2", "be2", "w_fc", "b_fc", "w_cp", "b_cp")))
    key = ("full", any_bias)
    if key not in _NC_CACHE:
        _NC_CACHE[key] = build_decoder_nc(with_bias=any_bias)
    nc = _NC_CACHE[key]

    in_maps = []
    for b in range(N_CORES):
        m = dict(arrs)
        m["xT"] = np.ascontiguousarray(x[b].T.astype(ml_dtypes.bfloat16))
        in_maps.append(m)

    from concourse.bass_utils import run_bass_kernel_spmd
    res = run_bass_kernel_spmd(nc, in_maps, list(range(N_CORES)))
    out = np.stack([res.results[i]["outT"].T.astype(np.float32)
                    for i in range(N_CORES)])
    return out



# revision 87
# speedup vs baseline: 1.0292x; 1.0108x over previous
"""Trainium2 Bass kernel for nn.DecoderBlock (pre-LN GPT block).

Shapes: B=8, T=1024, D=768, H=12, HD=64, F=3072.  Data-parallel: batch
element b runs on core b (no collectives needed).  All activations are
feature-major on chip ([D, T]: features on partitions, tokens free) so
chained matmuls need no transposes; attention scores are computed
transposed (S^T[t, q]) so softmax-weighted probabilities land directly in
the layout the P@V matmul needs.

Scheduling (the performance core of this kernel — ~1.6x over the naive
ordering of the same matmuls):
  * No GpSimd anywhere: partition broadcasts (LN apply, softmax
    denominator) are K=1 rank-1 PE matmuls into PSUM; causal masks are
    DVE multiplies on the exp output.  GpSimd semaphore+op latency
    (~1.3us per op) previously serialized the whole attention phase.
  * One ACT exp instruction per key-chunk covers BOTH heads of a
    128-partition pair via a two-bank PSUM score tile [128, 2, TQ]
    (halves ACT instruction-overhead; ACT is the attention-phase
    co-bottleneck with the PE).
  * P@V matmuls are software-pipelined two steps behind the score
    matmuls, with next-pair QK-projection matmuls split into 2-matmul
    micro-chunks pumped between steps, so the exp latency never idles
    the PE (PE idle gaps > ~3.4us re-throttle the PE clock to 1.2GHz —
    HAM — which was worth ~170us in the naive schedule).
  * V for ALL heads is computed upfront in 8 token-chunk units with the
    weight matrix as the moving operand (N=512/256 instead of N=128).
  * MLP is software-pipelined: ph(fc+1) matmuls are emitted before
    cp(fc), so each gelu runs under the next block of matmuls; final
    residual adds + output DMAs interleave into the last cp group.
  * LayerNorm: squares and PSUM evacuations on DVE; per-token stat
    chains run 128-lane in token-major layout (PE transposes); the
    apply uses rank-1 PE broadcasts.  All four ln_finish chains are
    generators whose PE ops pump between independent matmuls (stats(1)
    covers finish(0) at startup; V/QK units cover finish(1); the
    proj(1) units cover LN2's finish(0); pre-MLP fc units cover LN2's
    finish(1)), so their DVE serial chains never idle the PE.
  * LN2 runs entirely under the attention/proj tail: tci0 stats are
    last-pair filler, tci0's finish pumps into proj(1), tci1 mu AND
    sumsq stats interleave per proj(1) unit (separate PSUM banks so
    both groups stay open), and the first four fc units of the MLP run
    in the attention PSUM pools while finish(1) completes.
  * Several stat rows share one PSUM bank (matmul start=True clears
    only the bank's has_written bits, not its data), so the whole LN
    stat machinery needs just 3 banks across both layernorms.
  * Softmax normalization: the per-query denominator comes free as a
    65th ones-column in the P@V matmul; it is broadcast across
    partitions by a rank-1 PE matmul and reciprocated at full 128-lane
    DVE width after the broadcast (a [1,1024] single-lane recip is
    ~3x slower).
  * Host-side: LN affines and the 1/sqrt(HD) score scale are folded
    into the weights; weights are packed into DMA-contiguous lhsT
    tiles; matmuls run in bf16 with fp32 PSUM accumulation (fp8 was
    measured in simulation to breach the 2e-2 error budget: e4m3
    rounding alone contributes ~2.1% on the fc matmul).
  * Softmax max-subtraction is skipped: post-LN scores are O(5) so fp32
    exp cannot overflow.
  * The whole residual stream is bf16: x arrives bf16 from the host
    (kills the startup f32 DMA + on-chip cast; halves input DMA), X1 is
    bf16 (kills the LN2 scratch copies), the output leaves bf16 and is
    upcast on the host (halves output DMA, shortens the tail).
  * rstd = 1/sqrt(var+eps) runs entirely on DVE (bit-trick seed + 1
    Newton step, ~0.18% worst-case) instead of ACT Ln+Exp: each of the
    4 ln_finish calls previously triggered TWO ~1.3us ACT table loads
    on the critical path; now only the exp set (preloaded by a dummy at
    t=0) and the gelu set (preloaded right after proj(1)) are ever
    loaded, both off the PE critical path.
  * HAM warmup: ~100 junk matmuls on a memset tile run during the
    ~13us x-DMA wait so the LN1 stats chain starts at 2.4GHz instead
    of the cold 1.2GHz clock.
  * Softmax normalization of each (pair, qc) is deferred until the
    next score batch is in the PE queue, so its DVE work (den2 evac,
    recip, YT scale) overlaps scores instead of gating them.
  * Final residual adds are folded into the cp PSUM accumulation as
    identity matmuls; the output evacuations alternate DVE/ACT and the
    bf16 output DMA is half the size, shortening the post-last-matmul
    tail to ~6us.
"""

import numpy as np
import ml_dtypes

import concourse.bass as bass
import concourse.mybir as mybir
import concourse.tile as tile
from concourse import bacc

BF16 = mybir.dt.bfloat16
F32 = mybir.dt.float32
AF = mybir.ActivationFunctionType
OP = mybir.AluOpType

B, T, D, H = 8, 1024, 768, 12
HD = D // H
F = 4 * D
EPS = 1e-5
N_CORES = 8

KC = D // 128   # 6  contraction chunks over D
MC = D // 128   # 6  output-feature chunks over D
FC = F // 128   # 24 chunks over MLP hidden
NT = T // 128   # 8  key chunks
TS = 512        # token chunk (LN/proj/MLP)
TQ = 512        # query chunk
NQ = T // TQ    # 2
NJ = TS // 128  # 4
HPC = 2         # heads per 128-partition group
NPAIR = H // HPC  # 6
VS = HD + 1     # V columns per head incl ones-column (denominator row)


def build_decoder_nc(with_bias=False, eps=EPS, gelu_func=AF.Gelu_apprx_tanh):
    scale = 1.0  # 1/sqrt(HD) folded into wq host-side

    nc = bacc.Bacc()

    xT = nc.dram_tensor("xT", [D, T], BF16, kind="ExternalInput")
    wq_d = nc.dram_tensor("wq", [MC, 128, KC, 128], BF16, kind="ExternalInput")
    wk_d = nc.dram_tensor("wk", [MC, 128, KC, 128], BF16, kind="ExternalInput")
    wv_d = nc.dram_tensor("wv", [128, KC, D], BF16, kind="ExternalInput")
    wp_d = nc.dram_tensor("wp", [MC, 128, KC, 128], BF16, kind="ExternalInput")
    wf_d = nc.dram_tensor("wf", [FC, 128, KC, 128], BF16, kind="ExternalInput")
    wc_d = nc.dram_tensor("wc", [FC, 128, MC, 128], BF16, kind="ExternalInput")
    bias_d = {}
    if with_bias:
        for nm, width in (("bq", D), ("bk", D), ("bv", D), ("bp", D),
                          ("bf", F), ("bc", D)):
            bias_d[nm] = nc.dram_tensor(nm, [1, width], BF16,
                                        kind="ExternalInput")
    outT = nc.dram_tensor("outT", [D, T], BF16, kind="ExternalOutput")
    outT_t = outT[:].rearrange("(o p) t -> p o t", p=128)

    # ---- constants ----
    ones_bf = nc.inline_tensor(np.ones((1, T), ml_dtypes.bfloat16), "ones_bf")
    onescol_f = nc.inline_tensor(np.ones((128, 1), np.float32), "onescol_f")
    onescol_b = nc.inline_tensor(np.ones((128, 1), ml_dtypes.bfloat16),
                                 "onescol_b")
    onesrow_f = nc.inline_tensor(np.ones((1, 128), np.float32), "onesrow_f")
    onesrow_b = nc.inline_tensor(np.ones((1, 128), ml_dtypes.bfloat16),
                                 "onesrow_b")
    ident_b = nc.inline_tensor(np.eye(128, dtype=ml_dtypes.bfloat16),
                               "ident_b")
    # sel2[h, m] = 1 if m // 64 == h  (half-select broadcast)
    sel2_np = (np.arange(128)[None, :] // 64 ==
               np.arange(2)[:, None]).astype(np.float32)
    sel2_c = nc.inline_tensor(sel2_np, "sel2_c")
    # sel4[k, m] = 1 if m // 128 == k  (ln-stat row-select broadcast)
    sel4_np = (np.arange(512)[None, :] // 128 ==
               np.arange(4)[:, None]).astype(ml_dtypes.bfloat16)
    sel4_c = nc.inline_tensor(sel4_np, "sel4_c")
    ident_c = nc.inline_tensor(np.eye(128, dtype=np.float32), "ident_c")
    # multiplicative causal mask for transposed diagonal blocks: 1 if t <= q
    m_np = (np.arange(128)[:, None] <= np.arange(128)[None, :]).astype(
        ml_dtypes.bfloat16)
    masks_d = nc.inline_tensor(m_np, "masks")

    with tile.TileContext(nc) as tc:
        with (
            tc.tile_pool(name="persist", bufs=1) as pp,
            tc.tile_pool(name="wts", bufs=4) as wpool,
            tc.tile_pool(name="work", bufs=4) as wkp,
            tc.tile_pool(name="small", bufs=1) as sp,
        ):
            # ---- persistent SBUF ----
            ALN = pp.tile([128, KC, T], BF16, tag="ALN", name="ALN")
            QT = pp.tile([128, MC, T], BF16, tag="QT", name="QT")
            KT = pp.tile([128, MC, T], BF16, tag="KT", name="KT")
            Vt = pp.tile([128, NT, H * VS], BF16, tag="Vt", name="Vt")
            YT = pp.tile([128, KC, T], BF16, tag="YT", name="YT")
            X1 = pp.tile([128, KC, T], BF16, tag="X1", name="X1")
            A2 = pp.tile([128, KC, T], BF16, tag="A2", name="A2")
            wv_s = pp.tile([128, KC, D], BF16, tag="wv", name="wv_s")

            onescol_fs = pp.tile([128, 1], F32, tag="oc_f", name="onescol_fs")
            onescol_bs = pp.tile([128, 1], BF16, tag="oc_b", name="onescol_bs")
            onesrow_fs = pp.tile([1, 128], F32, tag="or_f", name="onesrow_fs")
            onesrow_bs = pp.tile([1, 128], BF16, tag="or_b", name="onesrow_bs")
            ident_bs = pp.tile([128, 128], BF16, tag="id_b", name="ident_bs")
            sel2_s = pp.tile([2, 128], F32, tag="sel2", name="sel2_s")
            sel4_s = pp.tile([4, 512], BF16, tag="sel4", name="sel4_s")
            ident_s = pp.tile([128, 128], F32, tag="ident", name="ident_s")
            masks_s = pp.tile([128, 128], BF16, tag="masks", name="masks_s")
            nc.sync.dma_start(out=onescol_fs, in_=onescol_f[:])
            nc.sync.dma_start(out=onescol_bs, in_=onescol_b[:])
            nc.sync.dma_start(out=onesrow_fs, in_=onesrow_f[:])
            nc.sync.dma_start(out=onesrow_bs, in_=onesrow_b[:])
            nc.sync.dma_start(out=ident_bs, in_=ident_b[:])
            nc.sync.dma_start(out=sel2_s, in_=sel2_c[:])
            nc.sync.dma_start(out=sel4_s, in_=sel4_c[:])
            nc.sync.dma_start(out=ident_s, in_=ident_c[:])
            nc.sync.dma_start(out=masks_s, in_=masks_d[:])
            onesb_s = None
            biases = {}
            if with_bias:
                onesb_s = pp.tile([1, T], BF16, tag="onesb", name="onesb_s")
                nc.sync.dma_start(out=onesb_s, in_=ones_bf[:])
                for nm, dten in bias_d.items():
                    bt = pp.tile(list(dten.shape), BF16, tag=nm,
                                 name=f"{nm}_s")
                    nc.sync.dma_start(out=bt, in_=dten[:])
                    biases[nm] = bt

            def bias_mm(psum, bias_t, msl, tsl):
                if bias_t is None:
                    return True
                nc.tensor.matmul(psum, bias_t[0:1, msl], onesb_s[0:1, tsl],
                                 start=True, stop=False)
                return False

            # ================= attention-phase PSUM pools =================
            with (
                tc.tile_pool(name="psA", bufs=2, space="PSUM") as psA,  # 4 banks
                tc.tile_pool(name="psB", bufs=1, space="PSUM") as psB,  # 2 banks
                tc.tile_pool(name="psC", bufs=2, space="PSUM") as psC,  # 2 banks
            ):
                # ---- load x^T (bf16 from host) BEFORE the V weights: the
                # LN1 stats chain gates everything, and per-kc full-T chunks
                # keep 2KB DMA lines (a tci-split halves line size and DMA
                # efficiency) ----
                xT_t = xT[:].rearrange("(o p) t -> p o t", p=128)
                for kc in range(KC):
                    nc.sync.dma_start(out=A2[:, kc, :], in_=xT_t[:, kc, :])
                nc.sync.dma_start(out=wv_s, in_=wv_d[:])

                # ---- LayerNorm (stats + apply); bf16 src, bf16 dst ----
                # row_mu / row_sq are [1, TS] PSUM APs.  A matmul start=True
                # zeroes the has_written bits of the whole 2KB bank, but not
                # the data, so many stat rows can share a bank as long as
                # their accumulation groups are strictly sequential.
                def ln_stats(sb, tci, row_mu, row_sq, sq, sqsl=None):
                    tsl = slice(tci * TS, (tci + 1) * TS)
                    for kc in range(KC):
                        nc.tensor.matmul(row_mu, onescol_bs[:],
                                         sb[:, kc, tsl],
                                         start=(kc == 0), stop=(kc == KC - 1))
                    for kc in range(KC):
                        nc.tensor.matmul(row_sq, onescol_bs[:],
                                         sq[:, kc, sqsl or slice(None)],
                                         start=(kc == 0), stop=(kc == KC - 1))

                def ln_finish_gen(src, dst, tci, row_mu, row_sq,
                                  act_evac=True, prep_pool=None,
                                  coarse_apply=False):
                    tsl = slice(tci * TS, (tci + 1) * TS)
                    srow_mu = sp.tile([1, TS], F32, tag="srow_mu",
                                      name="srow_mu")
                    srow_sq = sp.tile([1, TS], F32, tag="srow_sq",
                                      name="srow_sq")
                    # evacuate the two stat rows on different engines so they
                    # run concurrently (GpSimd cannot read PSUM); near the
                    # attention->MLP seam ACT is backlogged with exp/gelu, so
                    # those call sites use DVE for both
                    if act_evac:
                        nc.scalar.copy(out=srow_mu, in_=row_mu)
                    else:
                        nc.vector.tensor_copy(out=srow_mu, in_=row_mu)
                    nc.vector.tensor_copy(out=srow_sq, in_=row_sq)
                    yield
                    # token-major [128, NJ, 2] via PE transposes
                    ptk = psC.tile([128, NJ, 2], F32, tag="psC", name="ptk")
                    for jj in range(NJ):
                        jsl = slice(jj * 128, (jj + 1) * 128)
                        nc.tensor.transpose(
                            ptk[:, jj, 0:1], srow_mu[:, jsl],
                            ident_s[0:1, 0:1])
                        nc.tensor.transpose(
                            ptk[:, jj, 1:2], srow_sq[:, jsl],
                            ident_s[0:1, 0:1])
                        yield
                    stok = sp.tile([128, NJ, 2], F32, tag="stok", name="stok")
                    nc.vector.tensor_copy(out=stok, in_=ptk)
                    nc.vector.tensor_scalar_mul(stok, stok, 1.0 / D)
                    mu = stok[:, :, 0]
                    m2 = stok[:, :, 1]
                    var_t = sp.tile([128, NJ], F32, tag="var_t", name="var_t")
                    nc.vector.tensor_tensor(var_t, mu, mu, OP.mult)
                    nc.vector.tensor_tensor(var_t, m2, var_t, OP.subtract)
                    nc.vector.tensor_scalar_add(var_t, var_t, eps)
                    # st2[:,0,:]=rstd  st2[:,1,:]=-mu*rstd
                    # rstd = 1/sqrt(var+eps) entirely on DVE: bit-trick seed
                    # + 2 Newton steps.  No ACT Ln/Exp -> no ~1.3us ACT table
                    # loads on the LN critical path (2 per finish before).
                    st2 = sp.tile([128, 2, NJ], F32, tag="st2", name="st2")
                    y = st2[:, 0, :]
                    nr = sp.tile([128, NJ], F32, tag="nr_tmp", name="nr_tmp")
                    nc.vector.tensor_scalar(
                        y.bitcast(mybir.dt.int32),
                        var_t[:, :].bitcast(mybir.dt.int32),
                        1, None, OP.logical_shift_right)
                    # magic - t  ==  ~t + (magic + 1)  (two's complement);
                    # bitwise and arith ops can't share one instruction
                    nc.vector.tensor_scalar(
                        y.bitcast(mybir.dt.int32), y.bitcast(mybir.dt.int32),
                        0, None, OP.bitwise_not)
                    nc.vector.tensor_scalar(
                        y.bitcast(mybir.dt.int32), y.bitcast(mybir.dt.int32),
                        0x5f375a86 + 1, None, OP.add)
                    # one Newton step: <=0.18% rstd error, invisible next to
                    # the bf16 noise floor, and ~0.7us less serial DVE per
                    # ln_finish chain
                    for _ in range(1):
                        nc.vector.tensor_tensor(nr, y, y, OP.mult)
                        nc.vector.tensor_tensor(nr, nr, var_t, OP.mult)
                        nc.vector.tensor_scalar(nr, nr, -0.5, 1.5,
                                                OP.mult, OP.add)
                        nc.vector.tensor_tensor(y, y, nr, OP.mult)
                    nc.vector.tensor_tensor(st2[:, 1, :], mu, y, OP.mult)
                    nc.vector.tensor_scalar_mul(st2[:, 1, :], st2[:, 1, :],
                                                -1.0)
                    yield
                    # back to row layout per stat (all base-partition-0 APs)
                    prow = psC.tile([NJ, 2, 128], F32, tag="psC", name="prow")
                    nc.tensor.transpose(prow[:, 0, :], st2[:, 0, :],
                                        ident_s[:])
                    nc.tensor.transpose(prow[:, 1, :], st2[:, 1, :],
                                        ident_s[:])
                    yield
                    rows_sb = sp.tile([NJ, 2, 128], BF16, tag="rows_sb",
                                      name="rows_sb")
                    nc.vector.tensor_copy(out=rows_sb, in_=prow)
                    yield
                    # broadcast each 128-token stat row across partitions via
                    # a K=4 row-select matmul -- replaces a ~1.7us SBUF
                    # gather DMA that used to sit on this chain
                    pool = prep_pool if prep_pool is not None else psA
                    prep = pool.tile([128, 2, TS], F32, tag=pool.name,
                                     name="prep")
                    for jj in range(NJ):
                        jsl = slice(jj * 128, (jj + 1) * 128)
                        for sti in range(2):
                            nc.tensor.matmul(
                                prep[:, sti, jsl], sel4_s[:, jsl],
                                rows_sb[:, sti, :],
                                start=True, stop=True)
                    yield
                    tmp = wkp.tile([128, KC, TS], BF16, tag="lntmp", bufs=1,
                                   name="lntmp")
                    if coarse_apply:
                        # one whole-tci mult + add: consumers need the full
                        # half anyway, and 2 big DVE ops retire ~2x sooner
                        # than 8 per-jj ones (less per-op overhead + queue)
                        nc.vector.tensor_tensor(
                            tmp, src[:, :, tsl],
                            prep[:, 0:1, :].to_broadcast((128, KC, TS)),
                            OP.mult)
                        nc.vector.tensor_tensor(
                            dst[:, :, tsl], tmp,
                            prep[:, 1:2, :].to_broadcast((128, KC, TS)),
                            OP.add)
                        return
                    for jj in range(NJ):
                        jsl = slice(jj * 128, (jj + 1) * 128)
                        jtl = slice(tci * TS + jj * 128,
                                    tci * TS + (jj + 1) * 128)
                        # jj0 in two kc-halves: the first V/QK consumer can
                        # start on kc 0-2 ~1us before the full chunk lands
                        khs = ([slice(0, KC // 2), slice(KC // 2, KC)]
                               if jj == 0 else [slice(0, KC)])
                        for kh in khs:
                            nkc = kh.stop - kh.start
                            nc.vector.tensor_tensor(
                                tmp[:, kh, jsl], src[:, kh, jtl],
                                prep[:, 0:1, jsl].to_broadcast(
                                    (128, nkc, 128)), OP.mult)
                            nc.vector.tensor_tensor(
                                dst[:, kh, jtl], tmp[:, kh, jsl],
                                prep[:, 1:2, jsl].to_broadcast(
                                    (128, nkc, 128)), OP.add)

                def ln_finish(src, dst, tci, row_mu, row_sq, act_evac=True):
                    for _ in ln_finish_gen(src, dst, tci, row_mu, row_sq,
                                           act_evac):
                        pass

                # ---- V units (one per 128-token chunk, all heads) ----
                Vt4 = Vt.rearrange("p t (h c) -> p t h c", c=VS)
                nc.vector.memset(Vt4[:, :, :, HD:HD + 1], 1.0)

                def v_unit(tch):
                    t128 = slice(tch * 128, (tch + 1) * 128)
                    pvv = psA.tile([128, 2, TS], F32, tag="psA", name="pvv")
                    st0 = True
                    st1 = True
                    if with_bias:
                        nc.tensor.matmul(pvv[:, 0, :], onesb_s[0:1, 0:128],
                                         biases["bv"][0:1, 0:512],
                                         start=True, stop=False)
                        nc.tensor.matmul(pvv[:, 1, 0:256],
                                         onesb_s[0:1, 0:128],
                                         biases["bv"][0:1, 512:768],
                                         start=True, stop=False)
                        st0 = st1 = False
                    for kc in range(KC):
                        nc.tensor.matmul(
                            pvv[:, 0, :], ALN[:, kc, t128],
                            wv_s[:, kc, 0:512],
                            start=st0 and (kc == 0), stop=(kc == KC - 1))
                        nc.tensor.matmul(
                            pvv[:, 1, 0:256], ALN[:, kc, t128],
                            wv_s[:, kc, 512:768],
                            start=st1 and (kc == 0), stop=(kc == KC - 1))
                    # split the evacuation across ACT and DVE
                    nc.scalar.copy(
                        out=Vt4[:, tch, 0:8, 0:HD],
                        in_=pvv[:, 0, :].rearrange("p (h c) -> p h c", c=HD))
                    nc.vector.tensor_copy(
                        out=Vt4[:, tch, 8:12, 0:HD],
                        in_=pvv[:, 1, 0:256].rearrange("p (h c) -> p h c",
                                                       c=HD))

                # ---- QK unit generators (pair mc); yield per 2 matmuls ----
                def make_qk_gens(mc, tcis=(0, 1), on_act=False):
                    msl = slice(mc * 128, (mc + 1) * 128)
                    gens = []
                    wts = []
                    for nm, wten, dstT in (("bq", wq_d, QT), ("bk", wk_d, KT)):
                        wt = wpool.tile([128, KC, 128], BF16, tag="w_qk",
                                        bufs=4, name="wt_qk")
                        nc.sync.dma_start(out=wt, in_=wten[mc])
                        wts.append((nm, wt, dstT))
                    for tci in tcis:
                        for nm, wt, dstT in wts:
                            def qk_gen(nm=nm, wt=wt, dstT=dstT, tci=tci,
                                       on_act=on_act):
                                tsl = slice(tci * TS, (tci + 1) * TS)
                                pq = psC.tile([128, TS], F32, tag="psC",
                                              name="pq")
                                st = bias_mm(pq, biases.get(nm), msl, tsl)
                                for kc in range(KC):
                                    nc.tensor.matmul(
                                        pq, wt[:, kc, :], ALN[:, kc, tsl],
                                        start=st and (kc == 0),
                                        stop=(kc == KC - 1))
                                    if kc % 2 == 1 and kc < KC - 1:
                                        yield
                                if on_act:
                                    nc.scalar.copy(out=dstT[:, mc, tsl],
                                                   in_=pq)
                                else:
                                    nc.vector.tensor_copy(
                                        out=dstT[:, mc, tsl], in_=pq)
                            gens.append(qk_gen())
                    return gens

                # ---- proj unit generators (attn out-proj + residual) ----
                def make_proj_gens(tci):
                    tsl = slice(tci * TS, (tci + 1) * TS)
                    gens = []
                    for mc in range(MC):
                        wt = wpool.tile([128, KC, 128], BF16, tag="w_p",
                                        bufs=4, name="wt_p")
                        nc.sync.dma_start(out=wt, in_=wp_d[mc])

                        def proj_gen(mc=mc, wt=wt):
                            msl = slice(mc * 128, (mc + 1) * 128)
                            po = psC.tile([128, TS], F32, tag="psC", name="po")
                            st = bias_mm(po, biases.get("bp"), msl, tsl)
                            for kc in range(KC):
                                nc.tensor.matmul(
                                    po, wt[:, kc, :], YT[:, kc, tsl],
                                    start=st and (kc == 0), stop=False)
                                if kc % 2 == 1 and kc < KC - 1:
                                    yield
                            # residual folded into the PE accumulation
                            # (identity @ X); the X1 evacuation then runs on
                            # the idle ACT instead of the congested DVE
                            nc.tensor.matmul(
                                po, ident_bs[:], A2[:, mc, tsl],
                                start=False, stop=True)
                            nc.scalar.copy(out=X1[:, mc, tsl], in_=po)
                        gens.append(proj_gen(mc, wt))
                    return gens

                def pump(gens, n):
                    """Advance the generator queue by n yield-chunks."""
                    while n > 0 and gens:
                        try:
                            next(gens[0])
                        except StopIteration:
                            gens.pop(0)
                            continue
                        n -= 1

                def drain(gens):
                    for g in gens:
                        for _ in g:
                            pass
                    gens.clear()

                # ---- startup: LN1 overlapped with V and pair-0 QK ----
                # HAM warmup: the PE clock starts throttled at 1.2GHz and
                # only unthrottles after ~3.4us of sustained activity.  The
                # x DMA takes ~13us to land, so burn that wait on junk
                # matmuls (memset source: no DMA dependency) to both warm
                # the clock and keep it warm until the stats arrive.
                wu_src = sp.tile([128, 128], BF16, tag="wu_src", name="wu_src")
                nc.vector.memset(wu_src, 1.0)
                # trigger the exp table-set load NOW: it covers Copy too, so
                # ACT never loads a table again until the MLP gelu
                wu_act = sp.tile([1, 128], F32, tag="wu_act", name="wu_act")
                nc.scalar.activation(out=wu_act, in_=wu_src[0:1, :],
                                     func=AF.Exp)
                junk = psA.tile([128, 2, TQ], F32, tag="psA", name="junk")
                for _ in range(135):
                    nc.tensor.matmul(junk[:, 0, 0:128], wu_src[:],
                                     wu_src[:], start=True, stop=True)
                # x squares for BOTH token halves upfront: DVE computes them
                # chunk-by-chunk as the x DMA lands, while the PE is still on
                # warmup junk -- so the DVE is free later when the ln_finish
                # chains need it
                sq_all = wkp.tile([128, KC, T], BF16, tag="sq_all", bufs=1,
                                  name="sq_all")
                for kc in range(KC):
                    nc.vector.tensor_tensor(sq_all[:, kc, :], A2[:, kc, :],
                                            A2[:, kc, :], OP.mult)
                pstat1 = psB.tile([128, 2, TS], F32, tag="psB", name="pstat1")
                ln_stats(A2, 0, pstat1[0:1, 0, :], pstat1[0:1, 1, :],
                         sq_all, slice(0, TS))
                # finish(0) must fully drain before any ALN(tci0) consumer
                # matmul is emitted (the in-order PE queue would deadlock);
                # the stats(1) matmuls are pumped in as its latency cover.
                fin0 = [ln_finish_gen(A2, ALN, 0, pstat1[0:1, 0, :],
                                      pstat1[0:1, 1, :])]
                pump(fin0, 1)  # stat-row evacuations start now
                ln_stats(A2, 1, pstat1[64:65, 0, :], pstat1[64:65, 1, :],
                         sq_all, slice(TS, T))
                drain(fin0)
                # fin1's prep goes in psB (pstat1's slot, dead after its own
                # evacuations): in psA it would join the pvv rotation and
                # stall v_unit(4) on the whole LN1 apply(1)
                qk0_a = make_qk_gens(0, tcis=(0,), on_act=True)
                qk0_a.append(ln_finish_gen(A2, ALN, 1, pstat1[64:65, 0, :],
                                           pstat1[64:65, 1, :],
                                           prep_pool=psB))
                for tch in range(4):
                    v_unit(tch)
                    pump(qk0_a, 6)
                drain(qk0_a)
                qk0_b = make_qk_gens(0, tcis=(1,), on_act=True)
                for tch in range(4, NT):
                    v_unit(tch)
                    pump(qk0_b, 3)
                drain(qk0_b)
                # LN2(tci0) stats + finish as last-pair filler generators:
                # pumped after proj(0) drains, so apply(0) runs on DVE while
                # attention finishes and MLP can start right after proj(1).
                # pstat0 is allocated lazily (at first pump) so the psC
                # rotation during earlier pairs cannot clobber it.
                pstat0_box = []

                def stats0_gen():
                    pstat0_box.append(
                        psC.tile([128, TS], F32, tag="psC", name="pstat0"))
                    pstat0 = pstat0_box[0]
                    tsl0 = slice(0, TS)
                    sq = wkp.tile([128, KC, TS], BF16, tag="sq", bufs=1,
                                  name="sq")
                    for kc in range(KC):
                        nc.vector.tensor_tensor(
                            sq[:, kc, :], X1[:, kc, tsl0], X1[:, kc, tsl0],
                            OP.mult)
                    for kc in range(KC):
                        nc.tensor.matmul(pstat0[0:1, :], onescol_bs[:],
                                         X1[:, kc, tsl0],
                                         start=(kc == 0), stop=(kc == KC - 1))
                        if kc % 2 == 1 and kc < KC - 1:
                            yield
                    for kc in range(KC):
                        nc.tensor.matmul(pstat0[32:33, :], onescol_bs[:],
                                         sq[:, kc, :],
                                         start=(kc == 0), stop=(kc == KC - 1))
                        if kc % 2 == 1:
                            yield

                # finish(0)-of-LN2 starts inside the last-pair filler: its
                # stat-row evacuations then sit in the DVE queue AHEAD of the
                # last normalize/square batch instead of behind it
                fin0_box = []

                def fin0_start():
                    p0 = pstat0_box[0]
                    g = ln_finish_gen(X1, A2, 0, p0[0:1, :], p0[32:33, :],
                                      act_evac=False, coarse_apply=True)
                    fin0_box.append(g)
                    next(g)
                    yield

                norm_pend = []
                for mc in range(NPAIR):
                    last = mc + 1 >= NPAIR
                    filler = (make_proj_gens(0) if last
                              else make_qk_gens(mc + 1))
                    if last:
                        filler.append(stats0_gen())
                        filler.append(fin0_start())
                    for qc in range(NQ):
                        qsl = slice(qc * TQ, (qc + 1) * TQ)
                        ntch = (qc + 1) * (TQ // 128)
                        py = psB.tile([128, 2, TQ], F32, tag="psB", name="py")
                        pv_pend = []

                        def emit_pv(tch, pexp, rq, ntch=ntch, py=py, mc=mc):
                            for half in range(HPC):
                                h = mc * HPC + half
                                nc.tensor.matmul(
                                    py[0:VS, half, rq],
                                    Vt[:, tch, h * VS:(h + 1) * VS],
                                    pexp[:, half, rq],
                                    start=(tch == 0), stop=(tch == ntch - 1))

                        for tch in range(ntch):
                            t128 = slice(tch * 128, (tch + 1) * 128)
                            diag0 = qc * (TQ // 128)
                            dq = max(0, tch - diag0) * 128
                            rq = slice(dq, TQ)
                            qslr = slice(qc * TQ + dq, (qc + 1) * TQ)
                            psc = psA.tile([128, 2, TQ], F32, tag="psA",
                                           name="psc")
                            for half in range(HPC):
                                hsl = slice(half * HD, (half + 1) * HD)
                                nc.tensor.matmul(
                                    psc[:, half, rq], KT[hsl, mc, t128],
                                    QT[hsl, mc, qslr], start=True, stop=True)
                            # previous (pair, qc)'s softmax-normalize runs
                            # here, after this qc's first scores are already
                            # in the PE queue: its DVE work overlaps the PE
                            # instead of gating it at the qc boundary
                            if tch == 2 and norm_pend:
                                norm_pend.pop(0)()
                            pexp = wkp.tile([128, 2, TQ], BF16, tag="pexp",
                                            bufs=6, name="pexp")
                            nc.scalar.activation(out=pexp[:, :, rq],
                                                 in_=psc[:, :, rq],
                                                 func=AF.Exp)
                            if tch >= diag0:
                                nc.vector.tensor_tensor(
                                    pexp[:, :, dq:dq + 128],
                                    pexp[:, :, dq:dq + 128],
                                    masks_s[:, None, :].to_broadcast(
                                        (128, 2, 128)), OP.mult)
                            pv_pend.append((tch, pexp, rq))
                            # filler micro-chunks BEFORE the PV: the PE queue
                            # is in-order, so a PV stalled on its exp must
                            # not trap independent filler matmuls behind it
                            if qc == NQ - 1 or not last:
                                pump(filler, 4 if last else 1)
                            # software pipeline: PV three steps behind
                            # scores so the exp latency never stalls the PE
                            if len(pv_pend) > 4:
                                emit_pv(*pv_pend.pop(0))
                        if qc == NQ - 1 or not last:
                            pump(filler, 2)
                        while pv_pend:
                            emit_pv(*pv_pend.pop(0))

                        # softmax normalization (no GpSimd): both halves'
                        # denominators side by side in one base-0 row
                        def normalize(py=py, qsl=qsl, mc=mc):
                            den2 = sp.tile([1, 2 * TQ], BF16, tag="den2",
                                           name="den2")
                            for half in range(HPC):
                                nc.vector.tensor_copy(
                                    out=den2[:, half * TQ:(half + 1) * TQ],
                                    in_=py[HD:HD + 1, half, :])
                            prep1 = psC.tile([128, TS], F32, tag="psC",
                                             name="prep1")
                            for half in range(HPC):
                                nc.tensor.matmul(
                                    prep1[half * HD:(half + 1) * HD, :],
                                    onesrow_bs[0:1, 0:HD],
                                    den2[:, half * TQ:(half + 1) * TQ],
                                    start=True, stop=True)
                            # reciprocal AFTER the broadcast: 128-lane DVE op
                            # instead of a slow single-partition recip
                            prep_sb = wkp.tile([128, TS], F32, tag="prep_sb",
                                               bufs=2, name="prep_sb")
                            nc.vector.reciprocal_approx_fast(out=prep_sb,
                                                             in_=prep1)
                            for half in range(HPC):
                                hsl = slice(half * HD, (half + 1) * HD)
                                nc.vector.tensor_tensor(
                                    YT[hsl, mc, qsl], py[0:HD, half, :],
                                    prep_sb[hsl, :], OP.mult)

                        if last and qc == 0:
                            # proj(0) filler reads YT(tci0) this pair's qc=1:
                            # cannot defer
                            normalize()
                        else:
                            norm_pend.append(normalize)
                    drain(filler)
                while norm_pend:
                    norm_pend.pop(0)()

                # Prefetch the first MLP weight tiles now so their DMAs don't
                # queue behind the LN2 rows DMAs right when the MLP starts.
                pre_wts = []
                for fc in range(4):
                    wt = wpool.tile([128, KC, 128], BF16, tag="w_f",
                                    bufs=4, name="wt_f")
                    nc.sync.dma_start(out=wt, in_=wf_d[fc])
                    wtc = wpool.tile([128, MC, 128], BF16, tag="w_c",
                                     bufs=4, name="wt_c")
                    nc.sync.dma_start(out=wtc, in_=wc_d[fc])
                    pre_wts.append((wt, wtc))
                # LN2(tci0) stats ran as last-pair filler; finish(0) here so
                # apply(0) runs on DVE underneath proj(1).  LN2(tci1) mu AND
                # sumsq both interleave with the proj(1) units -- they
                # accumulate in different PSUM banks so both groups can stay
                # open across the loop.
                # emit finish(0)'s transposes + DVE-chain now, but leave its
                # later PE ops (prow/preps/apply) to pump between the proj(1)
                # units: the DVE chain then runs UNDER the proj matmuls
                # instead of idling the PE
                pump(fin0_box, 5)
                pstL2 = psB.tile([128, 2, TS], F32, tag="psB", name="pstL2")
                proj1 = make_proj_gens(1)
                tsl1 = slice(TS, 2 * TS)
                sq1 = wkp.tile([128, KC, TS], BF16, tag="sq1", bufs=1,
                               name="sq1")
                for mc, g in enumerate(proj1):
                    for _ in g:
                        pass
                    nc.vector.tensor_tensor(sq1[:, mc, :], X1[:, mc, tsl1],
                                            X1[:, mc, tsl1], OP.mult)
                    nc.tensor.matmul(pstL2[0:1, 0, :], onescol_bs[:],
                                     X1[:, mc, tsl1],
                                     start=(mc == 0), stop=(mc == KC - 1))
                    nc.tensor.matmul(pstL2[0:1, 1, :], onescol_bs[:],
                                     sq1[:, mc, :],
                                     start=(mc == 0), stop=(mc == KC - 1))
                    if mc >= 2:
                        pump(fin0_box, 2)
                drain(fin0_box)
                # trigger the gelu table-set load now: the proj X1
                # evacuations are done with ACT and the first real gelu
                # (pre-MLP units below) then hits a warm table.  Any earlier
                # and the 1.3us load delays a proj evacuation -> PE stall.
                nc.scalar.activation(out=wu_act, in_=wu_src[0:1, :],
                                     func=gelu_func)
                # Pre-MLP: the first two fc(qc=0) units run here in the
                # attention PSUM pools (they only need apply(0), done under
                # proj(1)), with finish(1) pumped between them, so the PE
                # never drains while LN2(tci1) finishes on DVE.
                fin1 = [ln_finish_gen(X1, A2, 1,
                                      pstL2[0:1, 0, :], pstL2[0:1, 1, :],
                                      act_evac=False, coarse_apply=True)]
                pump(fin1, 1)  # stat-row evacuations start now
                pre_hgel = []
                tsl0 = slice(0, TS)
                for fc in range(4):
                    wt, _wtc = pre_wts[fc]
                    ph = psA.tile([128, 2, TQ], F32, tag="psA",
                                  name="ph_pre")
                    st = bias_mm(ph[:, 0, 0:TS], biases.get("bf"),
                                 slice(fc * 128, (fc + 1) * 128), tsl0)
                    for kc in range(KC):
                        nc.tensor.matmul(
                            ph[:, 0, 0:TS], wt[:, kc, :], A2[:, kc, tsl0],
                            start=st and (kc == 0), stop=(kc == KC - 1))
                    hgel = wkp.tile([128, TS], BF16, tag="hgel", bufs=6,
                                    name="hgel")
                    nc.scalar.activation(out=hgel, in_=ph[:, 0, 0:TS],
                                         func=gelu_func)
                    pre_hgel.append(hgel)
                # both pre-MLP units are emitted BEFORE the rest of
                # finish(1): its PE ops wait on the DVE chain, and the fc
                # matmuls are the cover
                drain(fin1)

            # ================= MLP phase (new PSUM pools) =================
            with (
                tc.tile_pool(name="psPC", bufs=6, space="PSUM") as psPC,
                tc.tile_pool(name="psPH", bufs=2, space="PSUM") as psPH,
            ):
                for qc in range(T // TS):
                    tsl = slice(qc * TS, (qc + 1) * TS)
                    pcs = []
                    for mc in range(MC):
                        pc = psPC.tile([128, TS], F32, tag="psPC",
                                       name=f"pc{mc}")
                        st = bias_mm(pc, biases.get("bc"),
                                     slice(mc * 128, (mc + 1) * 128), tsl)
                        pcs.append((pc, st))
                    cp_pend = []
                    if qc == 0:
                        for fc in range(4):
                            cp_pend.append((fc, pre_hgel[fc], pre_wts[fc][1]))

                    def emit_cp(fc, hgel, wtc, pcs=pcs):
                        for mc in range(MC):
                            pc, st = pcs[mc]
                            nc.tensor.matmul(
                                pc, wtc[:, mc, :], hgel,
                                start=st and (fc == 0), stop=(fc == FC - 1))

                    for fc in range(len(cp_pend), FC):
                        fsl = slice(fc * 128, (fc + 1) * 128)
                        wt = wpool.tile([128, KC, 128], BF16, tag="w_f",
                                        bufs=4, name="wt_f")
                        nc.sync.dma_start(out=wt, in_=wf_d[fc])
                        wtc = wpool.tile([128, MC, 128], BF16, tag="w_c",
                                         bufs=4, name="wt_c")
                        nc.sync.dma_start(out=wtc, in_=wc_d[fc])
                        ph = psPH.tile([128, TS], F32, tag="psPH", name="ph")
                        st = bias_mm(ph, biases.get("bf"), fsl, tsl)
                        for kc in range(KC):
                            nc.tensor.matmul(
                                ph, wt[:, kc, :], A2[:, kc, tsl],
                                start=st and (kc == 0), stop=(kc == KC - 1))
                        hgel = wkp.tile([128, TS], BF16, tag="hgel", bufs=6,
                                        name="hgel")
                        nc.scalar.activation(out=hgel, in_=ph, func=gelu_func)
                        cp_pend.append((fc, hgel, wtc))
                        # deep pipeline: the first cp write of each qc lands
                        # on PSUM banks whose previous reader (LN2 apply /
                        # the other qc's residual adds, both on DVE) may
                        # still be draining
                        if len(cp_pend) > 4:
                            emit_cp(*cp_pend.pop(0))
                    while cp_pend:
                        fc_l, hgel_l, wtc_l = cp_pend.pop(0)
                        for mc in range(MC):
                            pc, st = pcs[mc]
                            nc.tensor.matmul(
                                pc, wtc_l[:, mc, :], hgel_l,
                                start=st and (fc_l == 0), stop=False)
                            if fc_l == FC - 1:
                                # residual add folded into the PE
                                # accumulation; the evacuation is then a
                                # plain copy that alternates ACT/DVE so the
                                # output tail drains on two engines
                                nc.tensor.matmul(
                                    pc, ident_bs[:], X1[:, mc, tsl],
                                    start=False, stop=True)
                                ot = wkp.tile([128, TS], BF16, tag="ot",
                                              bufs=4, name="ot")
                                if mc % 2 == 0:
                                    nc.vector.tensor_copy(out=ot, in_=pc[:])
                                else:
                                    nc.scalar.copy(out=ot, in_=pc[:])
                                nc.sync.dma_start(out=outT_t[:, mc, tsl],
                                                  in_=ot)

    nc.finalize()
    return nc


# --------------------------------------------------------------------------
# Host-side input prep
# --------------------------------------------------------------------------
def _pack_lhsT(w):
    """[Dk, N] -> [N//128, 128, Dk//128, 128] contiguous lhsT tiles."""
    Dk, N = w.shape
    return np.ascontiguousarray(
        w.reshape(Dk // 128, 128, N // 128, 128).transpose(2, 1, 0, 3))


def prepare_weights(wq, bq, wk, bk, wv, bv, w_proj, b_proj, g1, be1, g2, be2,
                    w_fc, b_fc, w_cp, b_cp):
    """Fold LN affines + 1/sqrt(HD) + reshape heads; packed bf16 arrays."""
    bf = ml_dtypes.bfloat16
    H_, D_, HD_ = wq.shape
    qscale = 1.0 / np.sqrt(HD_)
    wq2 = wq.transpose(1, 0, 2).reshape(D_, H_ * HD_).astype(np.float64)
    wk2 = wk.transpose(1, 0, 2).reshape(D_, H_ * HD_).astype(np.float64)
    wv2 = wv.transpose(1, 0, 2).reshape(D_, H_ * HD_).astype(np.float64)
    g1 = g1.astype(np.float64); be1 = be1.astype(np.float64)
    g2 = g2.astype(np.float64); be2 = be2.astype(np.float64)
    w_fc64 = w_fc.astype(np.float64)
    arrs = {
        "wq": _pack_lhsT((qscale * g1[:, None] * wq2).astype(bf)),
        "wk": _pack_lhsT((g1[:, None] * wk2).astype(bf)),
        "wv": np.ascontiguousarray(
            (g1[:, None] * wv2).astype(bf)
            .reshape(-1, 128, wv2.shape[1]).transpose(1, 0, 2)),
        "wp": _pack_lhsT(w_proj.astype(bf)),
        "wf": _pack_lhsT((g2[:, None] * w_fc64).astype(bf)),
        "wc": np.ascontiguousarray(
            w_cp.astype(bf).reshape(-1, 128, w_cp.shape[1] // 128, 128)),
    }
    bias_arrs = {
        "bq": (bq.reshape(-1).astype(np.float64) + be1 @ wq2) * qscale,
        "bk": bk.reshape(-1).astype(np.float64) + be1 @ wk2,
        "bv": bv.reshape(-1).astype(np.float64) + be1 @ wv2,
        "bp": b_proj.astype(np.float64),
        "bf": b_fc.astype(np.float64) + be2 @ w_fc64,
        "bc": b_cp.astype(np.float64),
    }
    any_bias = bool(any(np.any(v != 0) for v in bias_arrs.values()))
    if any_bias:
        for k, v in bias_arrs.items():
            arrs[k] = v.astype(bf).reshape(1, -1)
    return arrs, any_bias


_NC_CACHE = {}


def kernel(**inputs):
    x = np.asarray(inputs["x"], np.float32)
    arrs, any_bias = prepare_weights(
        *(np.asarray(inputs[k]) for k in (
            "wq", "bq", "wk", "bk", "wv", "bv", "w_proj", "b_proj",
            "g1", "be1", "g2", "be2", "w_fc", "b_fc", "w_cp", "b_cp")))
    key = ("full", any_bias)
    if key not in _NC_CACHE:
        _NC_CACHE[key] = build_decoder_nc(with_bias=any_bias)
    nc = _NC_CACHE[key]

    in_maps = []
    for b in range(N_CORES):
        m = dict(arrs)
        m["xT"] = np.ascontiguousarray(x[b].T.astype(ml_dtypes.bfloat16))
        in_maps.append(m)

    from concourse.bass_utils import run_bass_kernel_spmd
    res = run_bass_kernel_spmd(nc, in_maps, list(range(N_CORES)))
    out = np.stack([res.results[i]["outT"].T.astype(np.float32)
                    for i in range(N_CORES)])
    return out

